# revision 33
# baseline (speedup 1.0000x reference)
"""Trainium2 Bass kernel for nn_EruSelfAttentionModel (B=4,S=1024,E=1024,A=64,H=16,L=2).

Sharding: 8 cores; core c handles batch c//2 and heads (c%2)*8..(c%2)*8+8.
Heads are independent through both layers, so each core runs its (batch,
8-head) slice end-to-end with no collectives.

v3 (this file), on top of the bf16 v2 baseline -- all matmul chains run at
the PE streaming floor (~213ns per 512-col matmul), so the wins are fewer
columns and tighter overlap:

  * qk chains contract fp8e4 operands (weights shipped pre-quantized from
    the host).  Layer 1 reads a fp8 copy of hn0; layer 2 reads an
    UNCENTERED fp8 copy of the raw layer-1 output (kappa_s = 1/8 scaled)
    with the mean-subtraction folded in as a K=1 bf16 matmul appended to
    the same PSUM accumulation group (lhsT = -sum_e Wqk, rhs = mu row).
    fp8 at K=128 streams at bf16 speed with FWL weight loads; DoubleRow
    was measured a wash here (no background-buffer LDWEIGHTS on this
    compiler build -- --enable-ldw-opt=true crashes walrus codegen).
  * scores: one K=128 FWL matmul per (tb, s-half) against kzT=(zeros|k)
    and kqT=(q-junk|q), built with two half-psum copies + one
    partition-shift DMA (no kT zero-memset of the baseline's junk rows).
  * exp writes wT directly in fp8e4 (out1 runs mixed bf16 x fp8; out2 runs
    fp8 x fp8).  The z trees sum the same fp8 values, so the deferred
    softmax normalization stays exactly consistent.
  * layer-1 LN stats (mean / sum-of-squares) are ones-matmul chains over
    the fp8 raw copy and its fp8 Square.
  * v1 / out1 / v2 matmuls stay bf16: quantizing v1 or the centered raw
    to fp8 pushes rel-err past the 2e-2 budget (the softmax-averaged
    signal is ~1/32 of operand scale and LN re-amplifies it).

Scheduling (engine queues are strictly in-order, so emission order IS the
schedule):
  * per tb: two v psum groups first, then the scores pair + exp -- the v
    matmuls cover the qk-copy/shift-DMA latency and the exp drain.
  * the layer-2 softmax z pipeline (tree -> ones-matmul replicate ->
    transpose -> 1/z) and the rstd finalization are emitted as callbacks
    interleaved into the NEXT head's out1/out2 PE streams, hiding their
    DVE<->PE ping-pong latency (was ~9us of PE idle per head).
  * rstd is computed in COLUMN form (reciprocal of the transposed sd
    columns: 203ns instead of a 3.3us full-row DVE reciprocal); the
    replicated-row form the q2-multiply needs is rebuilt with per-block
    PE transposes + K=1 ones-matmuls.

Deferred-scale bookkeeping (y = kappa_s * out1_raw is what fp8 stores):
  rstd_s = 1/sqrt(var(y) + EPS*(kappa_s*z)^2) = rstd_true/kappa_s.
  q2 psum = kappa_s*Wq^T(x-mu)  -> *rstd_s            = true q
  k2 psum = kappa_s*Wk^T(x-mu)  -> exp scale rstd_s/SCALE cancels kappa_s
  v2 psum = (x-mu)^T Wv (bf16)  -> * (rstd_s*kappa_s) = true v
"""

import math
import os
from contextlib import ExitStack

import numpy as np
import ml_dtypes

# The device path (bass2jax under axon) needs the axon PJRT backend; a
# JAX_PLATFORMS=cpu pin (common for running the jax reference) would break it.
if "JAX_PLATFORMS" in os.environ and "axon" not in os.environ["JAX_PLATFORMS"]:
    del os.environ["JAX_PLATFORMS"]

import bass_rust
from bass_rust import SyncInfo
import concourse.bass as bass
import concourse.mybir as mybir
import concourse.tile as tile
from concourse.bass_utils import run_bass_kernel_spmd
from concourse.masks import make_identity
import concourse.bass_utils as _bass_utils

# NOTE: walrus's --enable-ldw-opt=true crashes codegen (visitInstLdweights,
# CoreV3GenImpl.cpp:694) on this build, so the flag stays false.  DoubleRow
# matmuls therefore pay their 256-col LDWEIGHTS serially (~+190ns per MM,
# measured), which cancels the 2x ALU win at FD=512 -- all chains below use
# plain K=128 matmuls (fp8 operands run at bf16 speed with FWL-hidden
# weight loads).

B, S, E, A, H, L, V = 4, 1024, 1024, 64, 16, 2, 32000
EPS = 1e-5
SCALE = math.sqrt(E)
P = 128
KO = E // P       # 8 k-blocks over E
SB = S // P       # 8 s-blocks
NH = H // 2       # 8 heads per core
HALF = S // 2     # 512
TA = 2 * A        # 128 (packed q|k)
KAPS = 1.0 / 8.0  # fp8 scale for the raw layer-1 output copy
FP = mybir.dt.float32
BF = mybir.dt.bfloat16
F8 = mybir.dt.float8e4
AF = mybir.ActivationFunctionType
OP = mybir.AluOpType
DR = mybir.MatmulPerfMode.DoubleRow

_WID = [0]


def _legalize_multi_waits(nc, max_keep=1):
    """This walrus build accepts at most one sync-wait command per engine
    instruction; split extras into standalone EventSemaphore waits."""
    for f in nc.m.functions:
        for blk in f.blocks:
            out = []
            changed = False
            for inst in blk.instructions:
                si = inst.sync_info
                ow = list(si.on_wait) if si is not None else []
                if len(ow) > max_keep:
                    changed = True
                    for w in ow[:-max_keep]:
                        _WID[0] += 1
                        out.append(bass_rust.InstEventSemaphore(
                            name=f"WSPLIT-{_WID[0]}",
                            engine=inst.engine,
                            ins=[], outs=[],
                            sync_info=SyncInfo(on_wait=[w], on_update=[]),
                        ))
                    inst.sync_info = SyncInfo(on_wait=ow[-max_keep:],
                                              on_update=list(si.on_update))
                out.append(inst)
            if changed:
                blk.instructions = out


def _build_nc(g0_identity, g1_identity, legalize=True):
    nc = bass.Bass("TRN2")

    emb = nc.dram_tensor("emb", [V, E], BF, kind="ExternalInput")
    xidx = nc.dram_tensor("xidx", [S, 1], mybir.dt.int32, kind="ExternalInput")
    wqk8 = nc.dram_tensor("wqk8", [L, NH, E, TA], F8, kind="ExternalInput")
    wv = nc.dram_tensor("wv", [L, NH, E, E], BF, kind="ExternalInput")
    negw1 = nc.dram_tensor("negw1", [NH, TA], BF, kind="ExternalInput")
    g0 = nc.dram_tensor("g0", [E], FP, kind="ExternalInput")
    b0 = nc.dram_tensor("b0", [E], FP, kind="ExternalInput")
    g1 = nc.dram_tensor("g1", [E], FP, kind="ExternalInput")
    b1 = nc.dram_tensor("b1", [E], FP, kind="ExternalInput")
    out_d = nc.dram_tensor("out", [NH, S, E], FP, kind="ExternalOutput")

    with tile.TileContext(nc) as tc, ExitStack() as ctx:
        const = ctx.enter_context(tc.tile_pool(name="const", bufs=1))
        hn0p = ctx.enter_context(tc.tile_pool(name="hn0p", bufs=1))
        wqkp = ctx.enter_context(tc.tile_pool(name="wqkp", bufs=2))
        wvp = ctx.enter_context(tc.tile_pool(name="wvp", bufs=2))
        vp = ctx.enter_context(tc.tile_pool(name="vp", bufs=1))
        v2p = ctx.enter_context(tc.tile_pool(name="v2p", bufs=1))
        wt1p = ctx.enter_context(tc.tile_pool(name="wt1p", bufs=1))
        wt2p = ctx.enter_context(tc.tile_pool(name="wt2p", bufs=1))
        qkp = ctx.enter_context(tc.tile_pool(name="qkp", bufs=1))
        htp = ctx.enter_context(tc.tile_pool(name="htp", bufs=2))
        h8p = ctx.enter_context(tc.tile_pool(name="h8p", bufs=2))
        sqp = ctx.enter_context(tc.tile_pool(name="sqp", bufs=1))
        hnsp = ctx.enter_context(tc.tile_pool(name="hnsp", bufs=2))
        stp = ctx.enter_context(tc.tile_pool(name="stp", bufs=2))
        otp = ctx.enter_context(tc.tile_pool(name="otp", bufs=2))
        zsbp = ctx.enter_context(tc.tile_pool(name="zsbp", bufs=2))
        zlp = ctx.enter_context(tc.tile_pool(name="zlp", bufs=1))
        lnp = ctx.enter_context(tc.tile_pool(name="lnp", bufs=1))
        mup = ctx.enter_context(tc.tile_pool(name="mup", bufs=1))
        mbp = ctx.enter_context(tc.tile_pool(name="mbp", bufs=2))
        rstp = ctx.enter_context(tc.tile_pool(name="rstp", bufs=1))
        rcp = ctx.enter_context(tc.tile_pool(name="rcp", bufs=1))
        # PSUM budget (8 banks of 2KB): psS tag "s" 2x2KB (qk chains +
        # scores pairs, temporally disjoint), psB "big" 3x2KB, psZ "z"
        # 2x2KB + stage0 "pst" 1x1KB -> 15KB.
        psS = ctx.enter_context(tc.tile_pool(name="psS", bufs=2, space="PSUM"))
        psB = ctx.enter_context(tc.tile_pool(name="psB", bufs=3, space="PSUM"))
        psZ = ctx.enter_context(tc.tile_pool(name="psZ", bufs=2, space="PSUM"))

        identB = const.tile([P, P], BF)
        make_identity(nc, identB[:])
        identF = const.tile([P, P], FP)
        make_identity(nc, identF[:])
        onesB = const.tile([P, P], BF)     # 1.0, for the z reduction
        nc.vector.memset(onesB[:], 1.0)
        ones8 = const.tile([P, 2, P], F8)  # 1.0, for the DR stats chains
        nc.vector.memset(ones8[:], 1.0)
        eps_t = const.tile([P, 1], FP)
        nc.vector.memset(eps_t[:], EPS)
        negw_sb = const.tile([1, NH, TA], BF)
        nc.sync.dma_start(negw_sb[:], negw1.ap()[None, :, :])
        if not g0_identity:
            g0rep = const.tile([P, E], FP)
            b0rep = const.tile([P, E], FP)
            nc.sync.dma_start(g0rep[:], g0.ap()[None, :].to_broadcast([P, E]))
            nc.sync.dma_start(b0rep[:], b0.ap()[None, :].to_broadcast([P, E]))
        if not g1_identity:
            g1c_t = const.tile([P, KO], FP)
            b1c_t = const.tile([P, KO], FP)
            nc.sync.dma_start(g1c_t[:], g1.ap().rearrange("(ko p) -> p ko", p=P))
            nc.sync.dma_start(b1c_t[:], b1.ap().rearrange("(ko p) -> p ko", p=P))

        # ---------------- stage 0: embed + LN0 + transpose -> hn0T ----------
        hn0T = hn0p.tile([P, KO, S], BF, tag="hn0")    # [e_inner, e_outer, s]
        hn0T8 = hn0p.tile([P, KO, S], F8, tag="hn08")  # fp8 copy for qk1 DR
        for sb in range(SB):
            idxt = stp.tile([P, 1], mybir.dt.int32, tag="idx")
            nc.sync.dma_start(idxt[:], xidx[sb * P:(sb + 1) * P, :])
            h0sb = hnsp.tile([P, E], BF, tag="h0")
            nc.gpsimd.indirect_dma_start(
                out=h0sb[:], out_offset=None, in_=emb[:, :],
                in_offset=bass.IndirectOffsetOnAxis(ap=idxt[:, :1], axis=0),
            )
            stats = stp.tile([P, 2, 6], FP, tag="bnst")
            nc.vector.bn_stats(stats[:, 0, :], h0sb[:, 0:HALF])
            nc.vector.bn_stats(stats[:, 1, :], h0sb[:, HALF:S])
            mv = stp.tile([P, 2], FP, tag="bnmv")
            nc.vector.bn_aggr(mv[:], stats[:])
            sd = stp.tile([P, 1], FP, tag="sd")
            nc.scalar.activation(sd[:], mv[:, 1:2], AF.Sqrt, bias=eps_t[:])
            rstd = stp.tile([P, 1], FP, tag="rstd")
            nc.vector.reciprocal(rstd[:], sd[:])
            nc.vector.tensor_scalar(h0sb[:], h0sb[:], scalar1=mv[:, 0:1],
                                    scalar2=rstd[:], op0=OP.subtract,
                                    op1=OP.mult)
            if not g0_identity:
                nc.vector.tensor_tensor(h0sb[:], h0sb[:], g0rep[:], OP.mult)
                nc.vector.tensor_tensor(h0sb[:], h0sb[:], b0rep[:], OP.add)
            for eo in range(0, KO, 4):
                pst = psZ.tile([P, 4, P], BF, tag="pst", bufs=1)
                for j in range(4):
                    nc.tensor.transpose(pst[:, j, :],
                                        h0sb[:, (eo + j) * P:(eo + j + 1) * P],
                                        identB[:])
                nc.any.tensor_copy(hn0T[:, eo:eo + 4, sb * P:(sb + 1) * P],
                                   pst[:, :, :])
                nc.any.tensor_copy(hn0T8[:, eo:eo + 4, sb * P:(sb + 1) * P],
                                   pst[:, :, :])

        def z_tree(wT, sh, pfx):
            """Per-partition partial sums of the 8 wT t-blocks.  First
            level reads wT's dtype (fp8 ok) on DVE; upper levels bf16.
            pfx keeps the long-lived zb tiles of independent trees apart
            so a later tree's alloc never stalls the DVE queue."""
            ssl = slice(sh * HALF, (sh + 1) * HALF)
            t0 = zlp.tile([P, HALF], BF, tag="zt0", bufs=1)
            t1 = zlp.tile([P, HALF], BF, tag="zt1", bufs=1)
            t2 = zlp.tile([P, HALF], BF, tag="zt2", bufs=1)
            t3 = zlp.tile([P, HALF], BF, tag="zt3", bufs=1)
            nc.vector.tensor_tensor(t0[:], wT[:, 0, ssl], wT[:, 1, ssl], OP.add)
            nc.vector.tensor_tensor(t1[:], wT[:, 2, ssl], wT[:, 3, ssl], OP.add)
            nc.vector.tensor_tensor(t2[:], wT[:, 4, ssl], wT[:, 5, ssl], OP.add)
            nc.vector.tensor_tensor(t3[:], wT[:, 6, ssl], wT[:, 7, ssl], OP.add)
            nc.gpsimd.tensor_tensor(t0[:], t0[:], t1[:], OP.add)
            nc.gpsimd.tensor_tensor(t2[:], t2[:], t3[:], OP.add)
            zb = zlp.tile([P, HALF], BF, tag=f"zb{pfx}{sh}")
            nc.vector.tensor_tensor(zb[:], t0[:], t2[:], OP.add)
            return zb

        def z_replicated(zpart):
            """Cross-partition reduction of the bf16 z partial sums on the
            PE via a ones matmul (output replicated on all partitions)."""
            ps_z = psZ.tile([P, HALF], FP, tag="z")
            nc.tensor.matmul(ps_z[:], lhsT=onesB[:], rhs=zpart[:],
                             start=True, stop=True)
            return ps_z

        # ---------------- qk chain (fp8 DoubleRow) ---------------------------
        def qk_chain(layer, head, hn8, mu_b, rstd_t):
            """Build qkT [q(0:A) | k(A:P), S] and the partition-swapped
            kqT [k | q] for row-tiled scores.  layer 2: mean correction is
            a K=1 bf16 matmul; q rows get *rstd_s before the copy."""
            wqk_sb = wqkp.tile([P, KO, TA], F8, tag="wqk")
            nc.sync.dma_start(
                wqk_sb[:],
                wqk8.ap()[layer, head].rearrange("(ko p) m -> p ko m", p=P))
            kqT = qkp.tile([P, S], BF, tag=f"kqT{layer}")
            kzT = qkp.tile([P, S], BF, tag=f"kzT{layer}")
            nc.vector.memset(kzT[0:A, :], 0.0)
            for nb in range(2):
                nsl = slice(nb * HALF, (nb + 1) * HALF)
                ps_qk = psS.tile([P, HALF], FP, tag="s")
                for ko in range(KO):
                    nc.tensor.matmul(ps_qk[:],
                                     lhsT=wqk_sb[:, ko, :],
                                     rhs=hn8[:, ko, nsl],
                                     start=(ko == 0),
                                     stop=(ko == KO - 1 and mu_b is None))
                if mu_b is not None:
                    nc.tensor.matmul(ps_qk[:],
                                     lhsT=negw_sb[0:1, head, :],
                                     rhs=mu_b[0:1, nb, :],
                                     start=False, stop=True)
                if rstd_t is not None:
                    nc.vector.tensor_tensor(ps_qk[0:A, :], ps_qk[0:A, :],
                                            rstd_t[0:A, nb, :], OP.mult)
                # q lands in kqT's low rows (finite junk against kzT's
                # zeros), then a partition-shift DMA moves it to the high
                # rows where the k|q contraction expects it.
                nc.scalar.copy(kqT[0:A, nsl], ps_qk[0:A, :])
                nc.scalar.copy(kzT[A:P, nsl], ps_qk[A:P, :])
                nc.sync.dma_start(kqT[A:P, nsl], kqT[0:A, nsl])
            return kzT, kqT

        # ---------------- scores + exp + v (interleaved) ---------------------
        def scores_v(layer, head, qkT, kqT, hn, wT, v_sb, wv_sb,
                     rstdc32, rstdc):
            # qkT here is kzT = (zeros | k); kqT = (junk | q)
            """Per tb: two v psum groups FIRST (keeps the PE busy while the
            qkT copies / shift-DMAs / exp drain), then both s-halves'
            scores as two concurrent row-tiled K=64 matmuls + exp (fp8)."""
            for tb in range(SB):
                tsl = slice(tb * P, (tb + 1) * P)
                for nb in range(2):
                    nsl = slice(nb * HALF, (nb + 1) * HALF)
                    ps_v = psB.tile([P, HALF], FP, tag="big")
                    for ko in range(KO):
                        nc.tensor.matmul(ps_v[:],
                                         lhsT=hn[:, ko, tsl],
                                         rhs=wv_sb[:, ko, nsl],
                                         start=(ko == 0), stop=(ko == KO - 1))
                    if rstdc is not None:
                        if tb % 2 == 0:
                            nc.scalar.activation(v_sb[:, tb, nsl], ps_v[:],
                                                 AF.Identity,
                                                 scale=rstdc[:, tb:tb + 1])
                        else:
                            nc.vector.tensor_scalar(v_sb[:, tb, nsl], ps_v[:],
                                                    scalar1=rstdc[:, tb:tb + 1],
                                                    scalar2=None, op0=OP.mult)
                    else:
                        if tb % 2 == 0:
                            nc.scalar.copy(v_sb[:, tb, nsl], ps_v[:])
                        else:
                            nc.vector.tensor_copy(v_sb[:, tb, nsl], ps_v[:])
                ps_a = psS.tile([P, HALF], FP, tag="s")
                ps_b = psS.tile([P, HALF], FP, tag="s")
                nc.tensor.matmul(ps_a[:], lhsT=qkT[:, tsl],
                                 rhs=kqT[:, 0:HALF],
                                 start=True, stop=True)
                nc.tensor.matmul(ps_b[:], lhsT=qkT[:, tsl],
                                 rhs=kqT[:, HALF:S],
                                 start=True, stop=True)
                if rstdc32 is not None:
                    nc.scalar.activation(wT[:, tb, 0:HALF], ps_a[:], AF.Exp,
                                         scale=rstdc32[:, tb:tb + 1])
                    nc.scalar.activation(wT[:, tb, HALF:S], ps_b[:], AF.Exp,
                                         scale=rstdc32[:, tb:tb + 1])
                else:
                    nc.scalar.activation(wT[:, tb, 0:HALF], ps_a[:], AF.Exp,
                                         scale=float(1.0 / SCALE))
                    nc.scalar.activation(wT[:, tb, HALF:S], ps_b[:], AF.Exp,
                                         scale=float(1.0 / SCALE))

        # ---------------- layer-1 out + stats --------------------------------
        def out1_stats(head, wT1, v_sb, aux=None):
            """outT (raw, [E,S]) in bf16 + fp8 (kappa_s); DR stats chains;
            var/rstd_s DVE chain; in-place mean centering of the bf16 raw.
            aux maps a flat (sh*KO+ob) group index to a callback emitted
            before that group -- used to interleave the previous head's
            layer-2 z pipeline into this head's PE stream."""
            aux = aux or {}
            raw = htp.tile([P, KO, S], BF, tag="raw")
            raw8 = h8p.tile([P, KO, S], F8, tag="raw8")
            mu_t = mup.tile([P, 2, HALF], FP, tag="mu_t")
            mu_b = mbp.tile([1, 2, HALF], BF, tag="mu_b")
            sdc = rcp.tile([P, SB], FP, tag="sdc")
            for sh in range(2):
                ssl = slice(sh * HALF, (sh + 1) * HALF)
                sq8 = sqp.tile([P, KO, HALF], F8, tag="sq8")
                zpart = z_tree(wT1, sh, "a")   # DVE-only; runs under the MMs
                for ob in range(KO):
                    if sh * KO + ob in aux:
                        aux[sh * KO + ob]()
                    ps_o = psB.tile([P, HALF], FP, tag="big")
                    for tb in range(SB):
                        nc.tensor.matmul(ps_o[:],
                                         lhsT=v_sb[:, tb, ob * P:(ob + 1) * P],
                                         rhs=wT1[:, tb, ssl],
                                         start=(tb == 0), stop=(tb == SB - 1))
                    if ob % 2 == 0:
                        nc.scalar.copy(raw[:, ob, ssl], ps_o[:])
                        nc.vector.tensor_scalar(raw8[:, ob, ssl], ps_o[:],
                                                scalar1=float(KAPS),
                                                scalar2=None, op0=OP.mult)
                    else:
                        nc.vector.tensor_copy(raw[:, ob, ssl], ps_o[:])
                        nc.scalar.activation(raw8[:, ob, ssl], ps_o[:],
                                             AF.Identity, scale=float(KAPS))
                    nc.scalar.activation(sq8[:, ob, :], raw8[:, ob, ssl],
                                         AF.Square)
                ps_mu = psZ.tile([P, HALF], FP, tag="z")
                for ko in range(KO):
                    nc.tensor.matmul(ps_mu[:], lhsT=ones8[:, 0, :],
                                     rhs=raw8[:, ko, ssl],
                                     start=(ko == 0), stop=(ko == KO - 1))
                # mu_t: x-domain mean (for centering); mu_b: y-domain (bf16
                # row 0 for the layer-2 K=1 correction); muy: y-domain fp32.
                nc.scalar.mul(mu_t[:, sh, :], ps_mu[:], float(1.0 / (E * KAPS)))
                muy = lnp.tile([P, HALF], FP, tag="muy")
                nc.vector.tensor_scalar(muy[:], ps_mu[:],
                                        scalar1=float(1.0 / E), scalar2=None,
                                        op0=OP.mult)
                nc.vector.tensor_scalar(mu_b[0:1, sh, :], ps_mu[0:1, :],
                                        scalar1=float(1.0 / E), scalar2=None,
                                        op0=OP.mult)
                ps_sq = psZ.tile([P, HALF], FP, tag="z")
                for ko in range(KO):
                    nc.tensor.matmul(ps_sq[:], lhsT=ones8[:, 0, :],
                                     rhs=sq8[:, ko, :],
                                     start=(ko == 0), stop=(ko == KO - 1))
                ps_z = z_replicated(zpart)
                # z scaled by sqrt(eps)*kappa_s; eps*(kappa_s*z)^2 is then a
                # Square on the scalar engine.
                z_sb = zsbp.tile([P, HALF], FP, tag="zsb", bufs=1)
                nc.scalar.mul(z_sb[:], ps_z[:], float(math.sqrt(EPS) * KAPS))
                zq = lnp.tile([P, HALF], FP, tag="zq")
                nc.scalar.activation(zq[:], z_sb[:], AF.Square)
                var = lnp.tile([P, HALF], FP, tag="var")
                nc.vector.tensor_tensor(var[:], muy[:], muy[:], OP.mult)
                nc.vector.scalar_tensor_tensor(var[:], ps_sq[:],
                                               float(1.0 / E), var[:],
                                               op0=OP.mult, op1=OP.subtract)
                nc.vector.tensor_tensor(var[:], var[:], zq[:], OP.add)
                nc.scalar.activation(var[:], var[:], AF.Sqrt)
                # sd -> column form now; the reciprocal happens later on the
                # tiny [P, SB] column tile (203ns) instead of a 3.3us DVE
                # reciprocal of the full [P, HALF] row form.
                ps_t = psZ.tile([P, HALF], FP, tag="z")
                for sbb in range(4):
                    nc.tensor.transpose(ps_t[:, sbb * P:(sbb + 1) * P],
                                        var[:, sbb * P:(sbb + 1) * P],
                                        identF[:])
                    nc.vector.tensor_copy(
                        sdc[:, sh * 4 + sbb:sh * 4 + sbb + 1],
                        ps_t[:, sbb * P:sbb * P + 1])
                # mean-center the bf16 raw in place (*rstd deferred)
                for ob in range(KO):
                    eng = nc.vector if ob % 2 == 0 else nc.gpsimd
                    eng.tensor_tensor(raw[:, ob, ssl], raw[:, ob, ssl],
                                      mu_t[:, sh, :], OP.subtract)
            return raw, raw8, mu_b, sdc

        def rstd_finalize(sdc):
            """From the sd column tile: reciprocal (tiny), the exp2/v2 scale
            columns, and the replicated-row rstd_s for the q2 multiply (per
            s-block: PE transpose of one rstd column -> [1,128] row -> K=1
            ones-matmul replicates it to all 128 partitions).  Returned as
            interleave callbacks for the surrounding out2 stream."""
            st = {}

            def f_cols():
                rstdc = rcp.tile([P, SB], FP, tag="rstdc", name="rstdc")
                nc.vector.reciprocal(rstdc[:], sdc[:])
                rstdc32 = rcp.tile([P, SB], FP, tag="r32", name="rstdc32")
                nc.vector.tensor_scalar(rstdc32[:], rstdc[:],
                                        scalar1=float(1.0 / SCALE),
                                        scalar2=None, op0=OP.mult)
                rstdcv = rcp.tile([P, SB], FP, tag="rv", name="rstdcv")
                nc.vector.tensor_scalar(rstdcv[:], rstdc[:],
                                        scalar1=float(KAPS), scalar2=None,
                                        op0=OP.mult)
                rowb = rcp.tile([1, SB, P], BF, tag="rowb", name="rowb")
                for half in range(2):
                    ps_t = psZ.tile([P, HALF], FP, tag="z", name="psrow")
                    for j in range(4):
                        idx = half * 4 + j
                        nc.tensor.transpose(ps_t[0:1, j * P:(j + 1) * P],
                                            rstdc[:, idx:idx + 1], identF[:])
                        nc.vector.tensor_copy(rowb[0:1, idx, :],
                                              ps_t[0:1, j * P:(j + 1) * P])
                st.update(rstdc32=rstdc32, rstdcv=rstdcv, rowb=rowb)

            def f_rows():
                rstd_t = rstp.tile([P, 2, HALF], FP, tag="rstd_t",
                                   name="rstd_t")
                rowb = st["rowb"]
                for sh in range(2):
                    ps_r = psZ.tile([P, HALF], FP, tag="z", name="psrep")
                    for blk in range(4):
                        nc.tensor.matmul(ps_r[:, blk * P:(blk + 1) * P],
                                         lhsT=onesB[0:1, :],
                                         rhs=rowb[0:1, sh * 4 + blk, :],
                                         start=True, stop=True)
                    nc.scalar.copy(rstd_t[:, sh, :], ps_r[:, :])
                st["rstd_t"] = rstd_t

            return {2: f_cols, 6: f_rows}, st

        # ---------------- layer-2 out (fp8 DoubleRow) + z + DMA --------------
        # The z pipeline alternates DVE and PE work with long serial
        # latency; emitted standalone it idles the PE ~9us per head.  It is
        # split into three steps interleaved into the NEXT head's out1
        # groups (aux mechanism above); out2_mm then only needs invzc.
        def z2_steps(wT2):
            st = {}

            def s_tree():
                st["zp0"] = z_tree(wT2, 0, "b")
                st["zp1"] = z_tree(wT2, 1, "b")

            def s_rep():
                st["zc"] = stp.tile([P, SB], FP, tag="zc", name="zc2")
                for sh in range(2):
                    ps_z = z_replicated(st[f"zp{sh}"])
                    z_sb = zsbp.tile([P, HALF], FP, tag="zsb2",
                                      name=f"zsb2_{sh}")
                    nc.scalar.copy(z_sb[:], ps_z[:])
                    st[f"zsb{sh}"] = z_sb

            def s_cols():
                zc = st["zc"]
                for sh in range(2):
                    ps_t = psZ.tile([P, HALF], FP, tag="z")
                    for sbb in range(4):
                        nc.tensor.transpose(ps_t[:, sbb * P:(sbb + 1) * P],
                                            st[f"zsb{sh}"][:, sbb * P:(sbb + 1) * P],
                                            identF[:])
                        nc.vector.tensor_copy(
                            zc[:, sh * 4 + sbb:sh * 4 + sbb + 1],
                            ps_t[:, sbb * P:sbb * P + 1])
                invzc = stp.tile([P, SB], FP, tag="invzc",
                                  name="invzc2")
                nc.vector.reciprocal(invzc[:], zc[:])
                st["invzc"] = invzc

            return {0: s_tree, 4: s_rep, 9: s_cols}, st

        def out2_mm(head, wT2, v2_sb, st, aux=None):
            aux = aux or {}
            invzc = st["invzc"]
            for blk in range(SB):
                bsl = slice(blk * P, (blk + 1) * P)
                for nb in range(2):
                    if blk * 2 + nb in aux:
                        aux[blk * 2 + nb]()
                    nsl = slice(nb * HALF, (nb + 1) * HALF)
                    ps_o = psB.tile([P, HALF], FP, tag="big")
                    for tb in range(SB):
                        nc.tensor.matmul(ps_o[:],
                                         lhsT=wT2[:, tb, bsl],
                                         rhs=v2_sb[:, tb, nsl],
                                         start=(tb == 0), stop=(tb == SB - 1))
                    ot = otp.tile([P, HALF], FP, tag="ot")
                    nc.vector.tensor_scalar_mul(ot[:], ps_o[:],
                                                invzc[:, blk:blk + 1])
                    nc.sync.dma_start(out_d.ap()[head, bsl, nsl], ot[:])

        # ------------- per-head loop, layer-2 pipelined one head back --------
        def wv_fetch(layer, h):
            wv_sb = wvp.tile([P, KO, E], BF, tag="wv", name=f"wv{layer}_{h}")
            nc.sync.dma_start(
                wv_sb[:],
                wv.ap()[layer, h].rearrange("(ko p) o -> p ko o", p=P))
            return wv_sb

        def l1_mid(h, wv_sb):
            qkT, kqT = qk_chain(0, h, hn0T8, None, None)
            wT1 = wt1p.tile([P, SB, S], F8, tag="wT1")
            v_sb = vp.tile([P, SB, E], BF, tag="v")
            scores_v(0, h, qkT, kqT, hn0T, wT1, v_sb, wv_sb, None, None)
            return wT1, v_sb

        def l2_mid(h, st, rst, wv_sb):
            raw, raw8, mu_b, _sdc = st
            qkT2, kqT2 = qk_chain(1, h, raw8, mu_b, rst["rstd_t"])
            wT2 = wt2p.tile([P, SB, S], F8, tag="wT2")
            v2_sb = v2p.tile([P, SB, E], F8, tag="v2")
            scores_v(1, h, qkT2, kqT2, raw, wT2, v2_sb, wv_sb,
                     rst["rstdc32"], rst["rstdcv"])
            return wT2, v2_sb

        pending = None
        for h in range(NH):
            wT1, v_sb = l1_mid(h, wv_fetch(0, h))
            if pending is not None:
                ph, pst, prst = pending
                wT2, v2_sb = l2_mid(ph, pst, prst, wv_fetch(1, ph))
                aux, zst = z2_steps(wT2)
            else:
                aux = None
            st = out1_stats(h, wT1, v_sb, aux=aux)
            raux, rst = rstd_finalize(st[3])
            if pending is not None:
                out2_mm(ph, wT2, v2_sb, zst, aux=raux)
            else:
                # head 0: no out2 stream to interleave into
                for fn in raux.values():
                    fn()
            pending = (h, st, rst)
        # drain: last head's layer 2 (z pipeline latency exposed once)
        ph, pst, prst = pending
        wT2, v2_sb = l2_mid(ph, pst, prst, wv_fetch(1, ph))
        aux, zst = z2_steps(wT2)
        for fn in aux.values():
            fn()
        out2_mm(ph, wT2, v2_sb, zst)

    if legalize:
        _legalize_multi_waits(nc)
    return nc


_CACHE = {}


def _get_nc(g0_identity, g1_identity, legalize=True):
    key = (g0_identity, g1_identity, legalize)
    if key not in _CACHE:
        _CACHE[key] = _build_nc(g0_identity, g1_identity, legalize)
    return _CACHE[key]


def _prep_in_maps(x, emb, ln_gamma, ln_beta, Wq, Wk, Wv):
    x = np.asarray(x)
    bf = ml_dtypes.bfloat16
    f8 = ml_dtypes.float8_e4m3
    emb = np.ascontiguousarray(np.asarray(emb, dtype=np.float32).astype(bf))
    ln_gamma = np.asarray(ln_gamma, dtype=np.float32)
    ln_beta = np.asarray(ln_beta, dtype=np.float32)
    Wq = np.asarray(Wq, dtype=np.float32)
    Wk = np.asarray(Wk, dtype=np.float32)
    Wv = np.asarray(Wv, dtype=np.float32)

    # [L,H,E,2A] packed (WqT | WkT) in fp8e4; [L,H,E,E] = WvT in bf16
    wqkT = np.concatenate([Wq.transpose(0, 1, 3, 2), Wk.transpose(0, 1, 3, 2)],
                          axis=3)
    wqkT8 = np.clip(wqkT, -240, 240).astype(f8)
    wvT = Wv.transpose(0, 1, 3, 2).astype(bf)
    # layer-2 mean-correction row: -(sum_e Wq2 | sum_e Wk2), consistent
    # with the fp8 weights actually used in the matmul.
    w1 = -wqkT8[1].astype(np.float32).sum(axis=1).astype(bf)  # [H, 2A]

    in_maps = []
    for c in range(8):
        b = c // 2
        hs = (c % 2) * NH
        in_maps.append({
            "emb": emb,
            "xidx": np.ascontiguousarray(x[b].astype(np.int32).reshape(S, 1)),
            "wqk8": np.ascontiguousarray(wqkT8[:, hs:hs + NH]),
            "wv": np.ascontiguousarray(wvT[:, hs:hs + NH]),
            "negw1": np.ascontiguousarray(w1[hs:hs + NH]),
            "g0": np.ascontiguousarray(ln_gamma[0]),
            "b0": np.ascontiguousarray(ln_beta[0]),
            "g1": np.ascontiguousarray(ln_gamma[1]),
            "b1": np.ascontiguousarray(ln_beta[1]),
        })
    g0_id = bool(np.all(ln_gamma[0] == 1.0) and np.all(ln_beta[0] == 0.0))
    g1_id = bool(np.all(ln_gamma[1] == 1.0) and np.all(ln_beta[1] == 0.0))
    return in_maps, g0_id, g1_id


def run(inputs, trace=False, trace_cores=None):
    in_maps, g0_id, g1_id = _prep_in_maps(**inputs)
    nc = _get_nc(g0_id, g1_id)
    res = run_bass_kernel_spmd(nc, in_maps, core_ids=list(range(8)),
                               trace=trace, trace_cores=trace_cores)
    out = np.empty((B, H, S, E), dtype=np.float32)
    for c in range(8):
        out[c // 2, (c % 2) * NH:(c % 2) * NH + NH] = res.results[c]["out"]
    return out, res


def kernel(x, emb, ln_gamma, ln_beta, Wq, Wk, Wv):
    out, _ = run(dict(x=x, emb=emb, ln_gamma=ln_gamma, ln_beta=ln_beta,
                      Wq=Wq, Wk=Wk, Wv=Wv))
    return out


# revision 35
# speedup vs baseline: 1.0131x; 1.0131x over previous
"""Trainium2 Bass kernel for nn_EruSelfAttentionModel (B=4,S=1024,E=1024,A=64,H=16,L=2).

Sharding: 8 cores; core c handles batch c//2 and heads (c%2)*8..(c%2)*8+8.
Heads are independent through both layers, so each core runs its (batch,
8-head) slice end-to-end with no collectives.

v3 (this file), on top of the bf16 v2 baseline -- all matmul chains run at
the PE streaming floor (~213ns per 512-col matmul), so the wins are fewer
columns and tighter overlap:

  * qk chains contract fp8e4 operands (weights shipped pre-quantized from
    the host).  Layer 1 reads a fp8 copy of hn0; layer 2 reads an
    UNCENTERED fp8 copy of the raw layer-1 output (kappa_s = 1/8 scaled)
    with the mean-subtraction folded in as a K=1 bf16 matmul appended to
    the same PSUM accumulation group (lhsT = -sum_e Wqk, rhs = mu row).
    fp8 at K=128 streams at bf16 speed with FWL weight loads; DoubleRow
    was measured a wash here (no background-buffer LDWEIGHTS on this
    compiler build -- --enable-ldw-opt=true crashes walrus codegen).
  * scores: one K=128 FWL matmul per (tb, s-half) against kzT=(zeros|k)
    and kqT=(q-junk|q), built with two half-psum copies + one
    partition-shift DMA (no kT zero-memset of the baseline's junk rows).
  * exp writes wT directly in fp8e4 (out1 runs mixed bf16 x fp8; out2 runs
    fp8 x fp8).  The z trees sum the same fp8 values, so the deferred
    softmax normalization stays exactly consistent.
  * layer-1 LN stats (mean / sum-of-squares) are ones-matmul chains over
    the fp8 raw copy and its fp8 Square.
  * v1 / out1 / v2 matmuls stay bf16: quantizing v1 or the centered raw
    to fp8 pushes rel-err past the 2e-2 budget (the softmax-averaged
    signal is ~1/32 of operand scale and LN re-amplifies it).

Scheduling (engine queues are strictly in-order, so emission order IS the
schedule):
  * per tb: two v psum groups first, then the scores pair + exp -- the v
    matmuls cover the qk-copy/shift-DMA latency and the exp drain.
  * the layer-2 softmax z pipeline (tree -> ones-matmul replicate ->
    transpose -> 1/z) and the rstd finalization are emitted as callbacks
    interleaved into the NEXT head's out1/out2 PE streams, hiding their
    DVE<->PE ping-pong latency (was ~9us of PE idle per head).
  * rstd is computed in COLUMN form (reciprocal of the transposed sd
    columns: 203ns instead of a 3.3us full-row DVE reciprocal); the
    replicated-row form the q2-multiply needs is rebuilt with per-block
    PE transposes + K=1 ones-matmuls.

Deferred-scale bookkeeping (y = kappa_s * out1_raw is what fp8 stores):
  rstd_s = 1/sqrt(var(y) + EPS*(kappa_s*z)^2) = rstd_true/kappa_s.
  q2 psum = kappa_s*Wq^T(x-mu)  -> *rstd_s            = true q
  k2 psum = kappa_s*Wk^T(x-mu)  -> exp scale rstd_s/SCALE cancels kappa_s
  v2 psum = (x-mu)^T Wv (bf16)  -> * (rstd_s*kappa_s) = true v
"""

import math
import os
from contextlib import ExitStack

import numpy as np
import ml_dtypes

# The device path (bass2jax under axon) needs the axon PJRT backend; a
# JAX_PLATFORMS=cpu pin (common for running the jax reference) would break it.
if "JAX_PLATFORMS" in os.environ and "axon" not in os.environ["JAX_PLATFORMS"]:
    del os.environ["JAX_PLATFORMS"]

import bass_rust
from bass_rust import SyncInfo
import concourse.bass as bass
import concourse.mybir as mybir
import concourse.tile as tile
from concourse.bass_utils import run_bass_kernel_spmd
from concourse.masks import make_identity
import concourse.bass_utils as _bass_utils

# NOTE: walrus's --enable-ldw-opt=true crashes codegen (visitInstLdweights,
# CoreV3GenImpl.cpp:694) on this build, so the flag stays false.  DoubleRow
# matmuls therefore pay their 256-col LDWEIGHTS serially (~+190ns per MM,
# measured), which cancels the 2x ALU win at FD=512 -- all chains below use
# plain K=128 matmuls (fp8 operands run at bf16 speed with FWL-hidden
# weight loads).

B, S, E, A, H, L, V = 4, 1024, 1024, 64, 16, 2, 32000
EPS = 1e-5
SCALE = math.sqrt(E)
P = 128
KO = E // P       # 8 k-blocks over E
SB = S // P       # 8 s-blocks
NH = H // 2       # 8 heads per core
HALF = S // 2     # 512
TA = 2 * A        # 128 (packed q|k)
KAPS = 1.0 / 8.0  # fp8 scale for the raw layer-1 output copy
FP = mybir.dt.float32
BF = mybir.dt.bfloat16
F8 = mybir.dt.float8e4
AF = mybir.ActivationFunctionType
OP = mybir.AluOpType
DR = mybir.MatmulPerfMode.DoubleRow

_WID = [0]


def _legalize_multi_waits(nc, max_keep=1):
    """This walrus build accepts at most one sync-wait command per engine
    instruction; split extras into standalone EventSemaphore waits."""
    for f in nc.m.functions:
        for blk in f.blocks:
            out = []
            changed = False
            for inst in blk.instructions:
                si = inst.sync_info
                ow = list(si.on_wait) if si is not None else []
                if len(ow) > max_keep:
                    changed = True
                    for w in ow[:-max_keep]:
                        _WID[0] += 1
                        out.append(bass_rust.InstEventSemaphore(
                            name=f"WSPLIT-{_WID[0]}",
                            engine=inst.engine,
                            ins=[], outs=[],
                            sync_info=SyncInfo(on_wait=[w], on_update=[]),
                        ))
                    inst.sync_info = SyncInfo(on_wait=ow[-max_keep:],
                                              on_update=list(si.on_update))
                out.append(inst)
            if changed:
                blk.instructions = out


def _build_nc(g0_identity, g1_identity, legalize=True):
    nc = bass.Bass("TRN2")

    emb = nc.dram_tensor("emb", [V, E], BF, kind="ExternalInput")
    xidx = nc.dram_tensor("xidx", [S, 1], mybir.dt.int32, kind="ExternalInput")
    wqk8 = nc.dram_tensor("wqk8", [L, NH, E, TA], F8, kind="ExternalInput")
    wv = nc.dram_tensor("wv", [L, NH, E, E], BF, kind="ExternalInput")
    negw1 = nc.dram_tensor("negw1", [NH, TA], BF, kind="ExternalInput")
    g0 = nc.dram_tensor("g0", [E], FP, kind="ExternalInput")
    b0 = nc.dram_tensor("b0", [E], FP, kind="ExternalInput")
    g1 = nc.dram_tensor("g1", [E], FP, kind="ExternalInput")
    b1 = nc.dram_tensor("b1", [E], FP, kind="ExternalInput")
    out_d = nc.dram_tensor("out", [NH, S, E], FP, kind="ExternalOutput")

    with tile.TileContext(nc) as tc, ExitStack() as ctx:
        const = ctx.enter_context(tc.tile_pool(name="const", bufs=1))
        hn0p = ctx.enter_context(tc.tile_pool(name="hn0p", bufs=1))
        wqkp = ctx.enter_context(tc.tile_pool(name="wqkp", bufs=2))
        wvp = ctx.enter_context(tc.tile_pool(name="wvp", bufs=2))
        vp = ctx.enter_context(tc.tile_pool(name="vp", bufs=1))
        v2p = ctx.enter_context(tc.tile_pool(name="v2p", bufs=1))
        wt1p = ctx.enter_context(tc.tile_pool(name="wt1p", bufs=1))
        wt2p = ctx.enter_context(tc.tile_pool(name="wt2p", bufs=1))
        qkp = ctx.enter_context(tc.tile_pool(name="qkp", bufs=1))
        htp = ctx.enter_context(tc.tile_pool(name="htp", bufs=2))
        h8p = ctx.enter_context(tc.tile_pool(name="h8p", bufs=2))
        sqp = ctx.enter_context(tc.tile_pool(name="sqp", bufs=1))
        hnsp = ctx.enter_context(tc.tile_pool(name="hnsp", bufs=2))
        stp = ctx.enter_context(tc.tile_pool(name="stp", bufs=2))
        otp = ctx.enter_context(tc.tile_pool(name="otp", bufs=2))
        zsbp = ctx.enter_context(tc.tile_pool(name="zsbp", bufs=2))
        zlp = ctx.enter_context(tc.tile_pool(name="zlp", bufs=1))
        lnp = ctx.enter_context(tc.tile_pool(name="lnp", bufs=1))
        mup = ctx.enter_context(tc.tile_pool(name="mup", bufs=1))
        mbp = ctx.enter_context(tc.tile_pool(name="mbp", bufs=2))
        rstp = ctx.enter_context(tc.tile_pool(name="rstp", bufs=1))
        rcp = ctx.enter_context(tc.tile_pool(name="rcp", bufs=1))
        # PSUM budget (8 banks of 2KB): psS tag "s" 2x2KB (qk chains +
        # scores pairs, temporally disjoint), psB "big" 3x2KB, psZ "z"
        # 2x2KB + stage0 "pst" 1x1KB -> 15KB.
        psS = ctx.enter_context(tc.tile_pool(name="psS", bufs=2, space="PSUM"))
        psB = ctx.enter_context(tc.tile_pool(name="psB", bufs=3, space="PSUM"))
        psZ = ctx.enter_context(tc.tile_pool(name="psZ", bufs=2, space="PSUM"))

        identB = const.tile([P, P], BF)
        make_identity(nc, identB[:])
        identF = const.tile([P, P], FP)
        make_identity(nc, identF[:])
        onesB = const.tile([P, P], BF)     # 1.0, for the z reduction
        nc.vector.memset(onesB[:], 1.0)
        ones8 = const.tile([P, 2, P], F8)  # 1.0, for the DR stats chains
        nc.vector.memset(ones8[:], 1.0)
        eps_t = const.tile([P, 1], FP)
        nc.vector.memset(eps_t[:], EPS)
        negw_sb = const.tile([1, NH, TA], BF)
        nc.sync.dma_start(negw_sb[:], negw1.ap()[None, :, :])
        if not g0_identity:
            g0rep = const.tile([P, E], FP)
            b0rep = const.tile([P, E], FP)
            nc.sync.dma_start(g0rep[:], g0.ap()[None, :].to_broadcast([P, E]))
            nc.sync.dma_start(b0rep[:], b0.ap()[None, :].to_broadcast([P, E]))
        if not g1_identity:
            g1c_t = const.tile([P, KO], FP)
            b1c_t = const.tile([P, KO], FP)
            nc.sync.dma_start(g1c_t[:], g1.ap().rearrange("(ko p) -> p ko", p=P))
            nc.sync.dma_start(b1c_t[:], b1.ap().rearrange("(ko p) -> p ko", p=P))

        # ---------------- stage 0: embed + LN0 + transpose -> hn0T ----------
        hn0T = hn0p.tile([P, KO, S], BF, tag="hn0")    # [e_inner, e_outer, s]
        hn0T8 = hn0p.tile([P, KO, S], F8, tag="hn08")  # fp8 copy for qk1 DR
        for sb in range(SB):
            idxt = stp.tile([P, 1], mybir.dt.int32, tag="idx")
            nc.sync.dma_start(idxt[:], xidx[sb * P:(sb + 1) * P, :])
            h0sb = hnsp.tile([P, E], BF, tag="h0")
            nc.gpsimd.indirect_dma_start(
                out=h0sb[:], out_offset=None, in_=emb[:, :],
                in_offset=bass.IndirectOffsetOnAxis(ap=idxt[:, :1], axis=0),
            )
            stats = stp.tile([P, 2, 6], FP, tag="bnst")
            nc.vector.bn_stats(stats[:, 0, :], h0sb[:, 0:HALF])
            nc.vector.bn_stats(stats[:, 1, :], h0sb[:, HALF:S])
            mv = stp.tile([P, 2], FP, tag="bnmv")
            nc.vector.bn_aggr(mv[:], stats[:])
            sd = stp.tile([P, 1], FP, tag="sd")
            nc.scalar.activation(sd[:], mv[:, 1:2], AF.Sqrt, bias=eps_t[:])
            rstd = stp.tile([P, 1], FP, tag="rstd")
            nc.vector.reciprocal(rstd[:], sd[:])
            nc.vector.tensor_scalar(h0sb[:], h0sb[:], scalar1=mv[:, 0:1],
                                    scalar2=rstd[:], op0=OP.subtract,
                                    op1=OP.mult)
            if not g0_identity:
                nc.vector.tensor_tensor(h0sb[:], h0sb[:], g0rep[:], OP.mult)
                nc.vector.tensor_tensor(h0sb[:], h0sb[:], b0rep[:], OP.add)
            for eo in range(0, KO, 4):
                pst = psZ.tile([P, 4, P], BF, tag="pst", bufs=1)
                for j in range(4):
                    nc.tensor.transpose(pst[:, j, :],
                                        h0sb[:, (eo + j) * P:(eo + j + 1) * P],
                                        identB[:])
                nc.any.tensor_copy(hn0T[:, eo:eo + 4, sb * P:(sb + 1) * P],
                                   pst[:, :, :])
                nc.any.tensor_copy(hn0T8[:, eo:eo + 4, sb * P:(sb + 1) * P],
                                   pst[:, :, :])

        def z_tree(wT, sh, pfx):
            """Per-partition partial sums of the 8 wT t-blocks.  First
            level reads wT's dtype (fp8 ok) on DVE; upper levels bf16.
            pfx keeps the long-lived zb tiles of independent trees apart
            so a later tree's alloc never stalls the DVE queue."""
            ssl = slice(sh * HALF, (sh + 1) * HALF)
            t0 = zlp.tile([P, HALF], BF, tag="zt0", bufs=1)
            t1 = zlp.tile([P, HALF], BF, tag="zt1", bufs=1)
            t2 = zlp.tile([P, HALF], BF, tag="zt2", bufs=1)
            t3 = zlp.tile([P, HALF], BF, tag="zt3", bufs=1)
            nc.vector.tensor_tensor(t0[:], wT[:, 0, ssl], wT[:, 1, ssl], OP.add)
            nc.vector.tensor_tensor(t1[:], wT[:, 2, ssl], wT[:, 3, ssl], OP.add)
            nc.vector.tensor_tensor(t2[:], wT[:, 4, ssl], wT[:, 5, ssl], OP.add)
            nc.vector.tensor_tensor(t3[:], wT[:, 6, ssl], wT[:, 7, ssl], OP.add)
            nc.gpsimd.tensor_tensor(t0[:], t0[:], t1[:], OP.add)
            nc.gpsimd.tensor_tensor(t2[:], t2[:], t3[:], OP.add)
            zb = zlp.tile([P, HALF], BF, tag=f"zb{pfx}{sh}")
            nc.vector.tensor_tensor(zb[:], t0[:], t2[:], OP.add)
            return zb

        def z_replicated(zpart):
            """Cross-partition reduction of the bf16 z partial sums on the
            PE via a ones matmul (output replicated on all partitions)."""
            ps_z = psZ.tile([P, HALF], FP, tag="z")
            nc.tensor.matmul(ps_z[:], lhsT=onesB[:], rhs=zpart[:],
                             start=True, stop=True)
            return ps_z

        # ---------------- qk chain (fp8 DoubleRow) ---------------------------
        def qk_chain(layer, head, hn8, mu_b, rstd_t):
            """Build qkT [q(0:A) | k(A:P), S] and the partition-swapped
            kqT [k | q] for row-tiled scores.  layer 2: mean correction is
            a K=1 bf16 matmul; q rows get *rstd_s before the copy."""
            wqk_sb = wqkp.tile([P, KO, TA], F8, tag="wqk")
            nc.sync.dma_start(
                wqk_sb[:],
                wqk8.ap()[layer, head].rearrange("(ko p) m -> p ko m", p=P))
            kqT = qkp.tile([P, S], BF, tag=f"kqT{layer}")
            kzT = qkp.tile([P, S], BF, tag=f"kzT{layer}")
            nc.vector.memset(kzT[0:A, :], 0.0)
            for nb in range(2):
                nsl = slice(nb * HALF, (nb + 1) * HALF)
                ps_qk = psS.tile([P, HALF], FP, tag="s")
                for ko in range(KO):
                    nc.tensor.matmul(ps_qk[:],
                                     lhsT=wqk_sb[:, ko, :],
                                     rhs=hn8[:, ko, nsl],
                                     start=(ko == 0),
                                     stop=(ko == KO - 1 and mu_b is None))
                if mu_b is not None:
                    nc.tensor.matmul(ps_qk[:],
                                     lhsT=negw_sb[0:1, head, :],
                                     rhs=mu_b[0:1, nb, :],
                                     start=False, stop=True)
                if rstd_t is not None:
                    nc.vector.tensor_tensor(ps_qk[0:A, :], ps_qk[0:A, :],
                                            rstd_t[0:A, nb, :], OP.mult)
                # q lands in kqT's low rows (finite junk against kzT's
                # zeros), then a partition-shift DMA moves it to the high
                # rows where the k|q contraction expects it.
                nc.scalar.copy(kqT[0:A, nsl], ps_qk[0:A, :])
                nc.scalar.copy(kzT[A:P, nsl], ps_qk[A:P, :])
                nc.sync.dma_start(kqT[A:P, nsl], kqT[0:A, nsl])
            return kzT, kqT

        # ---------------- scores + exp + v (interleaved) ---------------------
        def scores_v(layer, head, qkT, kqT, hn, wT, v_sb, wv_sb,
                     rstdc32, rstdc):
            # qkT here is kzT = (zeros | k); kqT = (junk | q)
            """Per tb: two v psum groups FIRST (keeps the PE busy while the
            qkT copies / shift-DMAs / exp drain), then both s-halves'
            scores as two concurrent row-tiled K=64 matmuls + exp (fp8)."""
            for tb in range(SB):
                tsl = slice(tb * P, (tb + 1) * P)
                for nb in range(2):
                    nsl = slice(nb * HALF, (nb + 1) * HALF)
                    ps_v = psB.tile([P, HALF], FP, tag="big")
                    for ko in range(KO):
                        nc.tensor.matmul(ps_v[:],
                                         lhsT=hn[:, ko, tsl],
                                         rhs=wv_sb[:, ko, nsl],
                                         start=(ko == 0), stop=(ko == KO - 1))
                    if rstdc is not None:
                        nc.vector.tensor_scalar(v_sb[:, tb, nsl], ps_v[:],
                                                scalar1=rstdc[:, tb:tb + 1],
                                                scalar2=None, op0=OP.mult)
                    else:
                        nc.vector.tensor_copy(v_sb[:, tb, nsl], ps_v[:])
                ps_a = psS.tile([P, HALF], FP, tag="s")
                ps_b = psS.tile([P, HALF], FP, tag="s")
                nc.tensor.matmul(ps_a[:], lhsT=qkT[:, tsl],
                                 rhs=kqT[:, 0:HALF],
                                 start=True, stop=True)
                nc.tensor.matmul(ps_b[:], lhsT=qkT[:, tsl],
                                 rhs=kqT[:, HALF:S],
                                 start=True, stop=True)
                if rstdc32 is not None:
                    nc.scalar.activation(wT[:, tb, 0:HALF], ps_a[:], AF.Exp,
                                         scale=rstdc32[:, tb:tb + 1])
                    nc.scalar.activation(wT[:, tb, HALF:S], ps_b[:], AF.Exp,
                                         scale=rstdc32[:, tb:tb + 1])
                else:
                    nc.scalar.activation(wT[:, tb, 0:HALF], ps_a[:], AF.Exp,
                                         scale=float(1.0 / SCALE))
                    nc.scalar.activation(wT[:, tb, HALF:S], ps_b[:], AF.Exp,
                                         scale=float(1.0 / SCALE))

        # ---------------- layer-1 out + stats --------------------------------
        def out1_stats(head, wT1, v_sb, aux=None):
            """outT (raw, [E,S]) in bf16 + fp8 (kappa_s); DR stats chains;
            var/rstd_s DVE chain; in-place mean centering of the bf16 raw.
            aux maps a flat (sh*KO+ob) group index to a callback emitted
            before that group -- used to interleave the previous head's
            layer-2 z pipeline into this head's PE stream."""
            aux = aux or {}
            raw = htp.tile([P, KO, S], BF, tag="raw")
            raw8 = h8p.tile([P, KO, S], F8, tag="raw8")
            mu_t = mup.tile([P, 2, HALF], FP, tag="mu_t")
            mu_b = mbp.tile([1, 2, HALF], BF, tag="mu_b")
            sdc = rcp.tile([P, SB], FP, tag="sdc")
            for sh in range(2):
                ssl = slice(sh * HALF, (sh + 1) * HALF)
                sq8 = sqp.tile([P, KO, HALF], F8, tag="sq8")
                zpart = z_tree(wT1, sh, "a")   # DVE-only; runs under the MMs
                for ob in range(KO):
                    if sh * KO + ob in aux:
                        aux[sh * KO + ob]()
                    ps_o = psB.tile([P, HALF], FP, tag="big")
                    for tb in range(SB):
                        nc.tensor.matmul(ps_o[:],
                                         lhsT=v_sb[:, tb, ob * P:(ob + 1) * P],
                                         rhs=wT1[:, tb, ssl],
                                         start=(tb == 0), stop=(tb == SB - 1))
                    if ob % 2 == 0:
                        nc.scalar.copy(raw[:, ob, ssl], ps_o[:])
                        nc.vector.tensor_scalar(raw8[:, ob, ssl], ps_o[:],
                                                scalar1=float(KAPS),
                                                scalar2=None, op0=OP.mult)
                    else:
                        nc.vector.tensor_copy(raw[:, ob, ssl], ps_o[:])
                        nc.scalar.activation(raw8[:, ob, ssl], ps_o[:],
                                             AF.Identity, scale=float(KAPS))
                    nc.scalar.activation(sq8[:, ob, :], raw8[:, ob, ssl],
                                         AF.Square)
                ps_mu = psZ.tile([P, HALF], FP, tag="z")
                for ko in range(KO):
                    nc.tensor.matmul(ps_mu[:], lhsT=ones8[:, 0, :],
                                     rhs=raw8[:, ko, ssl],
                                     start=(ko == 0), stop=(ko == KO - 1))
                # mu_t: x-domain mean (for centering); mu_b: y-domain (bf16
                # row 0 for the layer-2 K=1 correction); muy: y-domain fp32.
                nc.scalar.mul(mu_t[:, sh, :], ps_mu[:], float(1.0 / (E * KAPS)))
                muy = lnp.tile([P, HALF], FP, tag="muy")
                nc.vector.tensor_scalar(muy[:], ps_mu[:],
                                        scalar1=float(1.0 / E), scalar2=None,
                                        op0=OP.mult)
                nc.vector.tensor_scalar(mu_b[0:1, sh, :], ps_mu[0:1, :],
                                        scalar1=float(1.0 / E), scalar2=None,
                                        op0=OP.mult)
                ps_sq = psZ.tile([P, HALF], FP, tag="z")
                for ko in range(KO):
                    nc.tensor.matmul(ps_sq[:], lhsT=ones8[:, 0, :],
                                     rhs=sq8[:, ko, :],
                                     start=(ko == 0), stop=(ko == KO - 1))
                ps_z = z_replicated(zpart)
                # z scaled by sqrt(eps)*kappa_s; eps*(kappa_s*z)^2 is then a
                # Square on the scalar engine.
                z_sb = zsbp.tile([P, HALF], FP, tag="zsb", bufs=1)
                nc.scalar.mul(z_sb[:], ps_z[:], float(math.sqrt(EPS) * KAPS))
                zq = lnp.tile([P, HALF], FP, tag="zq")
                nc.scalar.activation(zq[:], z_sb[:], AF.Square)
                var = lnp.tile([P, HALF], FP, tag="var")
                nc.vector.tensor_tensor(var[:], muy[:], muy[:], OP.mult)
                nc.vector.scalar_tensor_tensor(var[:], ps_sq[:],
                                               float(1.0 / E), var[:],
                                               op0=OP.mult, op1=OP.subtract)
                nc.vector.tensor_tensor(var[:], var[:], zq[:], OP.add)
                nc.scalar.activation(var[:], var[:], AF.Sqrt)
                # sd -> column form now; the reciprocal happens later on the
                # tiny [P, SB] column tile (203ns) instead of a 3.3us DVE
                # reciprocal of the full [P, HALF] row form.
                ps_t = psZ.tile([P, HALF], FP, tag="z")
                for sbb in range(4):
                    nc.tensor.transpose(ps_t[:, sbb * P:(sbb + 1) * P],
                                        var[:, sbb * P:(sbb + 1) * P],
                                        identF[:])
                    nc.vector.tensor_copy(
                        sdc[:, sh * 4 + sbb:sh * 4 + sbb + 1],
                        ps_t[:, sbb * P:sbb * P + 1])
                # mean-center the bf16 raw in place (*rstd deferred)
                for ob in range(KO):
                    eng = nc.vector if ob % 2 == 0 else nc.gpsimd
                    eng.tensor_tensor(raw[:, ob, ssl], raw[:, ob, ssl],
                                      mu_t[:, sh, :], OP.subtract)
            return raw, raw8, mu_b, sdc

        def rstd_finalize(sdc):
            """From the sd column tile: reciprocal (tiny), the exp2/v2 scale
            columns, and the replicated-row rstd_s for the q2 multiply (per
            s-block: PE transpose of one rstd column -> [1,128] row -> K=1
            ones-matmul replicates it to all 128 partitions).  Returned as
            interleave callbacks for the surrounding out2 stream."""
            st = {}

            def f_cols():
                rstdc = rcp.tile([P, SB], FP, tag="rstdc", name="rstdc")
                nc.vector.reciprocal(rstdc[:], sdc[:])
                rstdc32 = rcp.tile([P, SB], FP, tag="r32", name="rstdc32")
                nc.vector.tensor_scalar(rstdc32[:], rstdc[:],
                                        scalar1=float(1.0 / SCALE),
                                        scalar2=None, op0=OP.mult)
                rstdcv = rcp.tile([P, SB], FP, tag="rv", name="rstdcv")
                nc.vector.tensor_scalar(rstdcv[:], rstdc[:],
                                        scalar1=float(KAPS), scalar2=None,
                                        op0=OP.mult)
                rowb = rcp.tile([1, SB, P], BF, tag="rowb", name="rowb")
                for half in range(2):
                    ps_t = psZ.tile([P, HALF], FP, tag="z", name="psrow")
                    for j in range(4):
                        idx = half * 4 + j
                        nc.tensor.transpose(ps_t[0:1, j * P:(j + 1) * P],
                                            rstdc[:, idx:idx + 1], identF[:])
                        nc.vector.tensor_copy(rowb[0:1, idx, :],
                                              ps_t[0:1, j * P:(j + 1) * P])
                st.update(rstdc32=rstdc32, rstdcv=rstdcv, rowb=rowb)

            def f_rows():
                rstd_t = rstp.tile([P, 2, HALF], FP, tag="rstd_t",
                                   name="rstd_t")
                rowb = st["rowb"]
                for sh in range(2):
                    ps_r = psZ.tile([P, HALF], FP, tag="z", name="psrep")
                    for blk in range(4):
                        nc.tensor.matmul(ps_r[:, blk * P:(blk + 1) * P],
                                         lhsT=onesB[0:1, :],
                                         rhs=rowb[0:1, sh * 4 + blk, :],
                                         start=True, stop=True)
                    nc.scalar.copy(rstd_t[:, sh, :], ps_r[:, :])
                st["rstd_t"] = rstd_t

            return {2: f_cols, 6: f_rows}, st

        # ---------------- layer-2 out (fp8 DoubleRow) + z + DMA --------------
        # The z pipeline alternates DVE and PE work with long serial
        # latency; emitted standalone it idles the PE ~9us per head.  It is
        # split into three steps interleaved into the NEXT head's out1
        # groups (aux mechanism above); out2_mm then only needs invzc.
        def z2_steps(wT2):
            st = {}

            def s_tree():
                st["zp0"] = z_tree(wT2, 0, "b")
                st["zp1"] = z_tree(wT2, 1, "b")

            def s_rep():
                st["zc"] = stp.tile([P, SB], FP, tag="zc", name="zc2")
                for sh in range(2):
                    ps_z = z_replicated(st[f"zp{sh}"])
                    z_sb = zsbp.tile([P, HALF], FP, tag="zsb2",
                                      name=f"zsb2_{sh}")
                    nc.scalar.copy(z_sb[:], ps_z[:])
                    st[f"zsb{sh}"] = z_sb

            def s_cols():
                zc = st["zc"]
                for sh in range(2):
                    ps_t = psZ.tile([P, HALF], FP, tag="z")
                    for sbb in range(4):
                        nc.tensor.transpose(ps_t[:, sbb * P:(sbb + 1) * P],
                                            st[f"zsb{sh}"][:, sbb * P:(sbb + 1) * P],
                                            identF[:])
                        nc.vector.tensor_copy(
                            zc[:, sh * 4 + sbb:sh * 4 + sbb + 1],
                            ps_t[:, sbb * P:sbb * P + 1])
                invzc = stp.tile([P, SB], FP, tag="invzc",
                                  name="invzc2")
                nc.vector.reciprocal(invzc[:], zc[:])
                st["invzc"] = invzc

            return {0: s_tree, 4: s_rep, 9: s_cols}, st

        def out2_mm(head, wT2, v2_sb, st, aux=None):
            aux = aux or {}
            invzc = st["invzc"]
            for blk in range(SB):
                bsl = slice(blk * P, (blk + 1) * P)
                for nb in range(2):
                    if blk * 2 + nb in aux:
                        aux[blk * 2 + nb]()
                    nsl = slice(nb * HALF, (nb + 1) * HALF)
                    ps_o = psB.tile([P, HALF], FP, tag="big")
                    for tb in range(SB):
                        nc.tensor.matmul(ps_o[:],
                                         lhsT=wT2[:, tb, bsl],
                                         rhs=v2_sb[:, tb, nsl],
                                         start=(tb == 0), stop=(tb == SB - 1))
                    ot = otp.tile([P, HALF], FP, tag="ot")
                    if (blk + nb) % 2 == 0:
                        nc.scalar.activation(ot[:], ps_o[:], AF.Identity,
                                             scale=invzc[:, blk:blk + 1])
                    else:
                        nc.vector.tensor_scalar_mul(ot[:], ps_o[:],
                                                    invzc[:, blk:blk + 1])
                    nc.sync.dma_start(out_d.ap()[head, bsl, nsl], ot[:])

        # ------------- per-head loop, layer-2 pipelined one head back --------
        def wv_fetch(layer, h):
            wv_sb = wvp.tile([P, KO, E], BF, tag="wv", name=f"wv{layer}_{h}")
            nc.sync.dma_start(
                wv_sb[:],
                wv.ap()[layer, h].rearrange("(ko p) o -> p ko o", p=P))
            return wv_sb

        def l1_mid(h, wv_sb):
            qkT, kqT = qk_chain(0, h, hn0T8, None, None)
            wT1 = wt1p.tile([P, SB, S], F8, tag="wT1")
            v_sb = vp.tile([P, SB, E], BF, tag="v")
            scores_v(0, h, qkT, kqT, hn0T, wT1, v_sb, wv_sb, None, None)
            return wT1, v_sb

        def l2_mid(h, st, rst, wv_sb):
            raw, raw8, mu_b, _sdc = st
            qkT2, kqT2 = qk_chain(1, h, raw8, mu_b, rst["rstd_t"])
            wT2 = wt2p.tile([P, SB, S], F8, tag="wT2")
            v2_sb = v2p.tile([P, SB, E], F8, tag="v2")
            scores_v(1, h, qkT2, kqT2, raw, wT2, v2_sb, wv_sb,
                     rst["rstdc32"], rst["rstdcv"])
            return wT2, v2_sb

        pending = None
        for h in range(NH):
            with nc.named_scope(f"l1mid{h}"):
                wT1, v_sb = l1_mid(h, wv_fetch(0, h))
            if pending is not None:
                ph, pst, prst = pending
                with nc.named_scope(f"l2mid{ph}"):
                    wT2, v2_sb = l2_mid(ph, pst, prst, wv_fetch(1, ph))
                aux, zst = z2_steps(wT2)
            else:
                aux = None
            with nc.named_scope(f"o1st{h}"):
                st = out1_stats(h, wT1, v_sb, aux=aux)
            raux, rst = rstd_finalize(st[3])
            if pending is not None:
                with nc.named_scope(f"o2mm{ph}"):
                    out2_mm(ph, wT2, v2_sb, zst, aux=raux)
            else:
                # head 0: no out2 stream to interleave into
                for fn in raux.values():
                    fn()
            pending = (h, st, rst)
        # drain: last head's layer 2 (z pipeline latency exposed once)
        ph, pst, prst = pending
        wT2, v2_sb = l2_mid(ph, pst, prst, wv_fetch(1, ph))
        aux, zst = z2_steps(wT2)
        for fn in aux.values():
            fn()
        out2_mm(ph, wT2, v2_sb, zst)

    if legalize:
        _legalize_multi_waits(nc)
    return nc


_CACHE = {}


def _get_nc(g0_identity, g1_identity, legalize=True):
    key = (g0_identity, g1_identity, legalize)
    if key not in _CACHE:
        _CACHE[key] = _build_nc(g0_identity, g1_identity, legalize)
    return _CACHE[key]


def _prep_in_maps(x, emb, ln_gamma, ln_beta, Wq, Wk, Wv):
    x = np.asarray(x)
    bf = ml_dtypes.bfloat16
    f8 = ml_dtypes.float8_e4m3
    emb = np.ascontiguousarray(np.asarray(emb, dtype=np.float32).astype(bf))
    ln_gamma = np.asarray(ln_gamma, dtype=np.float32)
    ln_beta = np.asarray(ln_beta, dtype=np.float32)
    Wq = np.asarray(Wq, dtype=np.float32)
    Wk = np.asarray(Wk, dtype=np.float32)
    Wv = np.asarray(Wv, dtype=np.float32)

    # [L,H,E,2A] packed (WqT | WkT) in fp8e4; [L,H,E,E] = WvT in bf16
    wqkT = np.concatenate([Wq.transpose(0, 1, 3, 2), Wk.transpose(0, 1, 3, 2)],
                          axis=3)
    wqkT8 = np.clip(wqkT, -240, 240).astype(f8)
    wvT = Wv.transpose(0, 1, 3, 2).astype(bf)
    # layer-2 mean-correction row: -(sum_e Wq2 | sum_e Wk2), consistent
    # with the fp8 weights actually used in the matmul.
    w1 = -wqkT8[1].astype(np.float32).sum(axis=1).astype(bf)  # [H, 2A]

    in_maps = []
    for c in range(8):
        b = c // 2
        hs = (c % 2) * NH
        in_maps.append({
            "emb": emb,
            "xidx": np.ascontiguousarray(x[b].astype(np.int32).reshape(S, 1)),
            "wqk8": np.ascontiguousarray(wqkT8[:, hs:hs + NH]),
            "wv": np.ascontiguousarray(wvT[:, hs:hs + NH]),
            "negw1": np.ascontiguousarray(w1[hs:hs + NH]),
            "g0": np.ascontiguousarray(ln_gamma[0]),
            "b0": np.ascontiguousarray(ln_beta[0]),
            "g1": np.ascontiguousarray(ln_gamma[1]),
            "b1": np.ascontiguousarray(ln_beta[1]),
        })
    g0_id = bool(np.all(ln_gamma[0] == 1.0) and np.all(ln_beta[0] == 0.0))
    g1_id = bool(np.all(ln_gamma[1] == 1.0) and np.all(ln_beta[1] == 0.0))
    return in_maps, g0_id, g1_id


def run(inputs, trace=False, trace_cores=None):
    in_maps, g0_id, g1_id = _prep_in_maps(**inputs)
    nc = _get_nc(g0_id, g1_id)
    res = run_bass_kernel_spmd(nc, in_maps, core_ids=list(range(8)),
                               trace=trace, trace_cores=trace_cores)
    out = np.empty((B, H, S, E), dtype=np.float32)
    for c in range(8):
        out[c // 2, (c % 2) * NH:(c % 2) * NH + NH] = res.results[c]["out"]
    return out, res


def kernel(x, emb, ln_gamma, ln_beta, Wq, Wk, Wv):
    out, _ = run(dict(x=x, emb=emb, ln_gamma=ln_gamma, ln_beta=ln_beta,
                      Wq=Wq, Wk=Wk, Wv=Wv))
    return out


# revision 37
# speedup vs baseline: 1.0186x; 1.0054x over previous
"""Trainium2 Bass kernel for nn_EruSelfAttentionModel (B=4,S=1024,E=1024,A=64,H=16,L=2).

Sharding: 8 cores; core c handles batch c//2 and heads (c%2)*8..(c%2)*8+8.
Heads are independent through both layers, so each core runs its (batch,
8-head) slice end-to-end with no collectives.

v3 (this file), on top of the bf16 v2 baseline -- all matmul chains run at
the PE streaming floor (~213ns per 512-col matmul), so the wins are fewer
columns and tighter overlap:

  * qk chains contract fp8e4 operands (weights shipped pre-quantized from
    the host).  Layer 1 reads a fp8 copy of hn0; layer 2 reads an
    UNCENTERED fp8 copy of the raw layer-1 output (kappa_s = 1/8 scaled)
    with the mean-subtraction folded in as a K=1 bf16 matmul appended to
    the same PSUM accumulation group (lhsT = -sum_e Wqk, rhs = mu row).
    fp8 at K=128 streams at bf16 speed with FWL weight loads; DoubleRow
    was measured a wash here (no background-buffer LDWEIGHTS on this
    compiler build -- --enable-ldw-opt=true crashes walrus codegen).
  * scores: one K=128 FWL matmul per (tb, s-half) against kzT=(zeros|k)
    and kqT=(q-junk|q), built with two half-psum copies + one
    partition-shift DMA (no kT zero-memset of the baseline's junk rows).
  * exp writes wT directly in fp8e4 (out1 runs mixed bf16 x fp8; out2 runs
    fp8 x fp8).  The z trees sum the same fp8 values, so the deferred
    softmax normalization stays exactly consistent.
  * layer-1 LN stats (mean / sum-of-squares) are ones-matmul chains over
    the fp8 raw copy and its fp8 Square.
  * v1 / out1 / v2 matmuls stay bf16: quantizing v1 or the centered raw
    to fp8 pushes rel-err past the 2e-2 budget (the softmax-averaged
    signal is ~1/32 of operand scale and LN re-amplifies it).

Scheduling (engine queues are strictly in-order, so emission order IS the
schedule):
  * per tb: two v psum groups first, then the scores pair + exp -- the v
    matmuls cover the qk-copy/shift-DMA latency and the exp drain.
  * the layer-2 softmax z pipeline (tree -> ones-matmul replicate ->
    transpose -> 1/z) and the rstd finalization are emitted as callbacks
    interleaved into the NEXT head's out1/out2 PE streams, hiding their
    DVE<->PE ping-pong latency (was ~9us of PE idle per head).
  * rstd is computed in COLUMN form (reciprocal of the transposed sd
    columns: 203ns instead of a 3.3us full-row DVE reciprocal); the
    replicated-row form the q2-multiply needs is rebuilt with per-block
    PE transposes + K=1 ones-matmuls.

Deferred-scale bookkeeping (y = kappa_s * out1_raw is what fp8 stores):
  rstd_s = 1/sqrt(var(y) + EPS*(kappa_s*z)^2) = rstd_true/kappa_s.
  q2 psum = kappa_s*Wq^T(x-mu)  -> *rstd_s            = true q
  k2 psum = kappa_s*Wk^T(x-mu)  -> exp scale rstd_s/SCALE cancels kappa_s
  v2 psum = (x-mu)^T Wv (bf16)  -> * (rstd_s*kappa_s) = true v
"""

import math
import os
from contextlib import ExitStack

import numpy as np
import ml_dtypes

# The device path (bass2jax under axon) needs the axon PJRT backend; a
# JAX_PLATFORMS=cpu pin (common for running the jax reference) would break it.
if "JAX_PLATFORMS" in os.environ and "axon" not in os.environ["JAX_PLATFORMS"]:
    del os.environ["JAX_PLATFORMS"]

import bass_rust
from bass_rust import SyncInfo
import concourse.bass as bass
import concourse.mybir as mybir
import concourse.tile as tile
from concourse.bass_utils import run_bass_kernel_spmd
from concourse.masks import make_identity
import concourse.bass_utils as _bass_utils

# NOTE: walrus's --enable-ldw-opt=true crashes codegen (visitInstLdweights,
# CoreV3GenImpl.cpp:694) on this build, so the flag stays false.  DoubleRow
# matmuls therefore pay their 256-col LDWEIGHTS serially (~+190ns per MM,
# measured), which cancels the 2x ALU win at FD=512 -- all chains below use
# plain K=128 matmuls (fp8 operands run at bf16 speed with FWL-hidden
# weight loads).

B, S, E, A, H, L, V = 4, 1024, 1024, 64, 16, 2, 32000
EPS = 1e-5
SCALE = math.sqrt(E)
P = 128
KO = E // P       # 8 k-blocks over E
SB = S // P       # 8 s-blocks
NH = H // 2       # 8 heads per core
HALF = S // 2     # 512
TA = 2 * A        # 128 (packed q|k)
KAPS = 1.0 / 8.0  # fp8 scale for the raw layer-1 output copy
FP = mybir.dt.float32
BF = mybir.dt.bfloat16
F8 = mybir.dt.float8e4
AF = mybir.ActivationFunctionType
OP = mybir.AluOpType
DR = mybir.MatmulPerfMode.DoubleRow

_WID = [0]


def _legalize_multi_waits(nc, max_keep=1):
    """This walrus build accepts at most one sync-wait command per engine
    instruction; split extras into standalone EventSemaphore waits."""
    for f in nc.m.functions:
        for blk in f.blocks:
            out = []
            changed = False
            for inst in blk.instructions:
                si = inst.sync_info
                ow = list(si.on_wait) if si is not None else []
                if len(ow) > max_keep:
                    changed = True
                    for w in ow[:-max_keep]:
                        _WID[0] += 1
                        out.append(bass_rust.InstEventSemaphore(
                            name=f"WSPLIT-{_WID[0]}",
                            engine=inst.engine,
                            ins=[], outs=[],
                            sync_info=SyncInfo(on_wait=[w], on_update=[]),
                        ))
                    inst.sync_info = SyncInfo(on_wait=ow[-max_keep:],
                                              on_update=list(si.on_update))
                out.append(inst)
            if changed:
                blk.instructions = out


def _build_nc(g0_identity, g1_identity, legalize=True):
    nc = bass.Bass("TRN2")

    emb = nc.dram_tensor("emb", [V, E], BF, kind="ExternalInput")
    xidx = nc.dram_tensor("xidx", [S, 1], mybir.dt.int32, kind="ExternalInput")
    wqk8 = nc.dram_tensor("wqk8", [L, NH, E, TA], F8, kind="ExternalInput")
    wv = nc.dram_tensor("wv", [L, NH, E, E], BF, kind="ExternalInput")
    negw1 = nc.dram_tensor("negw1", [NH, TA], BF, kind="ExternalInput")
    g0 = nc.dram_tensor("g0", [E], FP, kind="ExternalInput")
    b0 = nc.dram_tensor("b0", [E], FP, kind="ExternalInput")
    g1 = nc.dram_tensor("g1", [E], FP, kind="ExternalInput")
    b1 = nc.dram_tensor("b1", [E], FP, kind="ExternalInput")
    out_d = nc.dram_tensor("out", [NH, S, E], FP, kind="ExternalOutput")

    with tile.TileContext(nc) as tc, ExitStack() as ctx:
        const = ctx.enter_context(tc.tile_pool(name="const", bufs=1))
        hn0p = ctx.enter_context(tc.tile_pool(name="hn0p", bufs=1))
        wqkp = ctx.enter_context(tc.tile_pool(name="wqkp", bufs=2))
        wvp = ctx.enter_context(tc.tile_pool(name="wvp", bufs=2))
        vp = ctx.enter_context(tc.tile_pool(name="vp", bufs=1))
        v2p = ctx.enter_context(tc.tile_pool(name="v2p", bufs=1))
        wt1p = ctx.enter_context(tc.tile_pool(name="wt1p", bufs=1))
        wt2p = ctx.enter_context(tc.tile_pool(name="wt2p", bufs=1))
        qkp = ctx.enter_context(tc.tile_pool(name="qkp", bufs=1))
        htp = ctx.enter_context(tc.tile_pool(name="htp", bufs=2))
        h8p = ctx.enter_context(tc.tile_pool(name="h8p", bufs=2))
        sqp = ctx.enter_context(tc.tile_pool(name="sqp", bufs=1))
        hnsp = ctx.enter_context(tc.tile_pool(name="hnsp", bufs=2))
        stp = ctx.enter_context(tc.tile_pool(name="stp", bufs=2))
        otp = ctx.enter_context(tc.tile_pool(name="otp", bufs=2))
        zsbp = ctx.enter_context(tc.tile_pool(name="zsbp", bufs=2))
        zlp = ctx.enter_context(tc.tile_pool(name="zlp", bufs=1))
        lnp = ctx.enter_context(tc.tile_pool(name="lnp", bufs=1))
        mup = ctx.enter_context(tc.tile_pool(name="mup", bufs=1))
        mbp = ctx.enter_context(tc.tile_pool(name="mbp", bufs=2))
        rstp = ctx.enter_context(tc.tile_pool(name="rstp", bufs=1))
        rcp = ctx.enter_context(tc.tile_pool(name="rcp", bufs=1))
        # PSUM budget (8 banks of 2KB): psS tag "s" 2x2KB (qk chains +
        # scores pairs, temporally disjoint), psB "big" 3x2KB, psZ "z"
        # 2x2KB + stage0 "pst" 1x1KB -> 15KB.
        psS = ctx.enter_context(tc.tile_pool(name="psS", bufs=2, space="PSUM"))
        psB = ctx.enter_context(tc.tile_pool(name="psB", bufs=3, space="PSUM"))
        psZ = ctx.enter_context(tc.tile_pool(name="psZ", bufs=2, space="PSUM"))

        identB = const.tile([P, P], BF)
        make_identity(nc, identB[:])
        identF = const.tile([P, P], FP)
        make_identity(nc, identF[:])
        onesB = const.tile([P, P], BF)     # 1.0, for the z reduction
        nc.vector.memset(onesB[:], 1.0)
        ones8 = const.tile([P, 2, P], F8)  # 1.0, for the DR stats chains
        nc.vector.memset(ones8[:], 1.0)
        eps_t = const.tile([P, 1], FP)
        nc.vector.memset(eps_t[:], EPS)
        negw_sb = const.tile([1, NH, TA], BF)
        nc.sync.dma_start(negw_sb[:], negw1.ap()[None, :, :])
        if not g0_identity:
            g0rep = const.tile([P, E], FP)
            b0rep = const.tile([P, E], FP)
            nc.sync.dma_start(g0rep[:], g0.ap()[None, :].to_broadcast([P, E]))
            nc.sync.dma_start(b0rep[:], b0.ap()[None, :].to_broadcast([P, E]))
        if not g1_identity:
            g1c_t = const.tile([P, KO], FP)
            b1c_t = const.tile([P, KO], FP)
            nc.sync.dma_start(g1c_t[:], g1.ap().rearrange("(ko p) -> p ko", p=P))
            nc.sync.dma_start(b1c_t[:], b1.ap().rearrange("(ko p) -> p ko", p=P))

        # ---------------- stage 0: embed + LN0 + transpose -> hn0T ----------
        hn0T = hn0p.tile([P, KO, S], BF, tag="hn0")    # [e_inner, e_outer, s]
        hn0T8 = hn0p.tile([P, KO, S], F8, tag="hn08")  # fp8 copy for qk1 DR
        for sb in range(SB):
            idxt = stp.tile([P, 1], mybir.dt.int32, tag="idx")
            nc.sync.dma_start(idxt[:], xidx[sb * P:(sb + 1) * P, :])
            h0sb = hnsp.tile([P, E], BF, tag="h0")
            nc.gpsimd.indirect_dma_start(
                out=h0sb[:], out_offset=None, in_=emb[:, :],
                in_offset=bass.IndirectOffsetOnAxis(ap=idxt[:, :1], axis=0),
            )
            stats = stp.tile([P, 2, 6], FP, tag="bnst")
            nc.vector.bn_stats(stats[:, 0, :], h0sb[:, 0:HALF])
            nc.vector.bn_stats(stats[:, 1, :], h0sb[:, HALF:S])
            mv = stp.tile([P, 2], FP, tag="bnmv")
            nc.vector.bn_aggr(mv[:], stats[:])
            sd = stp.tile([P, 1], FP, tag="sd")
            nc.scalar.activation(sd[:], mv[:, 1:2], AF.Sqrt, bias=eps_t[:])
            rstd = stp.tile([P, 1], FP, tag="rstd")
            nc.vector.reciprocal(rstd[:], sd[:])
            nc.vector.tensor_scalar(h0sb[:], h0sb[:], scalar1=mv[:, 0:1],
                                    scalar2=rstd[:], op0=OP.subtract,
                                    op1=OP.mult)
            if not g0_identity:
                nc.vector.tensor_tensor(h0sb[:], h0sb[:], g0rep[:], OP.mult)
                nc.vector.tensor_tensor(h0sb[:], h0sb[:], b0rep[:], OP.add)
            for eo in range(0, KO, 4):
                pst = psZ.tile([P, 4, P], BF, tag="pst", bufs=1)
                for j in range(4):
                    nc.tensor.transpose(pst[:, j, :],
                                        h0sb[:, (eo + j) * P:(eo + j + 1) * P],
                                        identB[:])
                nc.any.tensor_copy(hn0T[:, eo:eo + 4, sb * P:(sb + 1) * P],
                                   pst[:, :, :])
                nc.any.tensor_copy(hn0T8[:, eo:eo + 4, sb * P:(sb + 1) * P],
                                   pst[:, :, :])

        def z_tree(wT, sh, pfx):
            """Per-partition partial sums of the 8 wT t-blocks.  First
            level reads wT's dtype (fp8 ok) on DVE; upper levels bf16.
            pfx keeps the long-lived zb tiles of independent trees apart
            so a later tree's alloc never stalls the DVE queue."""
            ssl = slice(sh * HALF, (sh + 1) * HALF)
            t0 = zlp.tile([P, HALF], BF, tag="zt0", bufs=1)
            t1 = zlp.tile([P, HALF], BF, tag="zt1", bufs=1)
            t2 = zlp.tile([P, HALF], BF, tag="zt2", bufs=1)
            t3 = zlp.tile([P, HALF], BF, tag="zt3", bufs=1)
            nc.vector.tensor_tensor(t0[:], wT[:, 0, ssl], wT[:, 1, ssl], OP.add)
            nc.vector.tensor_tensor(t1[:], wT[:, 2, ssl], wT[:, 3, ssl], OP.add)
            nc.vector.tensor_tensor(t2[:], wT[:, 4, ssl], wT[:, 5, ssl], OP.add)
            nc.vector.tensor_tensor(t3[:], wT[:, 6, ssl], wT[:, 7, ssl], OP.add)
            nc.gpsimd.tensor_tensor(t0[:], t0[:], t1[:], OP.add)
            nc.gpsimd.tensor_tensor(t2[:], t2[:], t3[:], OP.add)
            zb = zlp.tile([P, HALF], BF, tag=f"zb{pfx}{sh}")
            nc.vector.tensor_tensor(zb[:], t0[:], t2[:], OP.add)
            return zb

        def z_replicated(zpart):
            """Cross-partition reduction of the bf16 z partial sums on the
            PE via a ones matmul (output replicated on all partitions)."""
            ps_z = psZ.tile([P, HALF], FP, tag="z")
            nc.tensor.matmul(ps_z[:], lhsT=onesB[:], rhs=zpart[:],
                             start=True, stop=True)
            return ps_z

        # ---------------- qk chain (fp8 DoubleRow) ---------------------------
        def qk_chain(layer, head, hn8, mu_b, rstd_t):
            """Build qkT [q(0:A) | k(A:P), S] and the partition-swapped
            kqT [k | q] for row-tiled scores.  layer 2: mean correction is
            a K=1 bf16 matmul; q rows get *rstd_s before the copy."""
            wqk_sb = wqkp.tile([P, KO, TA], F8, tag="wqk")
            nc.sync.dma_start(
                wqk_sb[:],
                wqk8.ap()[layer, head].rearrange("(ko p) m -> p ko m", p=P))
            kqT = qkp.tile([P, S], BF, tag=f"kqT{layer}")
            kzT = qkp.tile([P, S], BF, tag=f"kzT{layer}")
            nc.vector.memset(kzT[0:A, :], 0.0)
            for nb in range(2):
                nsl = slice(nb * HALF, (nb + 1) * HALF)
                ps_qk = psS.tile([P, HALF], FP, tag="s")
                for ko in range(KO):
                    nc.tensor.matmul(ps_qk[:],
                                     lhsT=wqk_sb[:, ko, :],
                                     rhs=hn8[:, ko, nsl],
                                     start=(ko == 0),
                                     stop=(ko == KO - 1 and mu_b is None))
                if mu_b is not None:
                    nc.tensor.matmul(ps_qk[:],
                                     lhsT=negw_sb[0:1, head, :],
                                     rhs=mu_b[0:1, nb, :],
                                     start=False, stop=True)
                if rstd_t is not None:
                    nc.vector.tensor_tensor(ps_qk[0:A, :], ps_qk[0:A, :],
                                            rstd_t[0:A, nb, :], OP.mult)
                # q lands in kqT's low rows (finite junk against kzT's
                # zeros), then a partition-shift DMA moves it to the high
                # rows where the k|q contraction expects it.
                nc.scalar.copy(kqT[0:A, nsl], ps_qk[0:A, :])
                nc.scalar.copy(kzT[A:P, nsl], ps_qk[A:P, :])
                nc.sync.dma_start(kqT[A:P, nsl], kqT[0:A, nsl])
            return kzT, kqT

        # ---------------- scores + exp + v (interleaved) ---------------------
        def scores_v(layer, head, qkT, kqT, hn, wT, v_sb, wv_sb,
                     rstdc32, rstdc):
            # qkT here is kzT = (zeros | k); kqT = (junk | q)
            """Per tb: two v psum groups FIRST (keeps the PE busy while the
            qkT copies / shift-DMAs / exp drain), then both s-halves'
            scores as two concurrent row-tiled K=64 matmuls + exp (fp8)."""
            for tb in range(SB):
                tsl = slice(tb * P, (tb + 1) * P)
                for nb in range(2):
                    nsl = slice(nb * HALF, (nb + 1) * HALF)
                    ps_v = psB.tile([P, HALF], FP, tag="big")
                    for ko in range(KO):
                        nc.tensor.matmul(ps_v[:],
                                         lhsT=hn[:, ko, tsl],
                                         rhs=wv_sb[:, ko, nsl],
                                         start=(ko == 0), stop=(ko == KO - 1))
                    if rstdc is not None:
                        if tb % 2 == 0:
                            nc.scalar.activation(v_sb[:, tb, nsl], ps_v[:],
                                                 AF.Identity,
                                                 scale=rstdc[:, tb:tb + 1])
                        else:
                            nc.vector.tensor_scalar(v_sb[:, tb, nsl], ps_v[:],
                                                    scalar1=rstdc[:, tb:tb + 1],
                                                    scalar2=None, op0=OP.mult)
                    else:
                        if tb % 2 == 0:
                            nc.scalar.copy(v_sb[:, tb, nsl], ps_v[:])
                        else:
                            nc.vector.tensor_copy(v_sb[:, tb, nsl], ps_v[:])
                ps_a = psS.tile([P, HALF], FP, tag="s")
                ps_b = psS.tile([P, HALF], FP, tag="s")
                nc.tensor.matmul(ps_a[:], lhsT=qkT[:, tsl],
                                 rhs=kqT[:, 0:HALF],
                                 start=True, stop=True)
                nc.tensor.matmul(ps_b[:], lhsT=qkT[:, tsl],
                                 rhs=kqT[:, HALF:S],
                                 start=True, stop=True)
                if rstdc32 is not None:
                    nc.scalar.activation(wT[:, tb, 0:HALF], ps_a[:], AF.Exp,
                                         scale=rstdc32[:, tb:tb + 1])
                    nc.scalar.activation(wT[:, tb, HALF:S], ps_b[:], AF.Exp,
                                         scale=rstdc32[:, tb:tb + 1])
                else:
                    nc.scalar.activation(wT[:, tb, 0:HALF], ps_a[:], AF.Exp,
                                         scale=float(1.0 / SCALE))
                    nc.scalar.activation(wT[:, tb, HALF:S], ps_b[:], AF.Exp,
                                         scale=float(1.0 / SCALE))

        # ---------------- layer-1 out + stats --------------------------------
        def out1_stats(head, wT1, v_sb, aux=None):
            """outT (raw, [E,S]) in bf16 + fp8 (kappa_s); DR stats chains;
            var/rstd_s DVE chain; in-place mean centering of the bf16 raw.
            aux maps a flat (sh*KO+ob) group index to a callback emitted
            before that group -- used to interleave the previous head's
            layer-2 z pipeline into this head's PE stream."""
            aux = aux or {}
            raw = htp.tile([P, KO, S], BF, tag="raw")
            raw8 = h8p.tile([P, KO, S], F8, tag="raw8")
            mu_t = mup.tile([P, 2, HALF], FP, tag="mu_t")
            mu_b = mbp.tile([1, 2, HALF], BF, tag="mu_b")
            sdc = rcp.tile([P, SB], FP, tag="sdc")
            for sh in range(2):
                ssl = slice(sh * HALF, (sh + 1) * HALF)
                sq8 = sqp.tile([P, KO, HALF], F8, tag="sq8")
                zpart = z_tree(wT1, sh, "a")   # DVE-only; runs under the MMs
                for ob in range(KO):
                    if sh * KO + ob in aux:
                        aux[sh * KO + ob]()
                    ps_o = psB.tile([P, HALF], FP, tag="big")
                    for tb in range(SB):
                        nc.tensor.matmul(ps_o[:],
                                         lhsT=v_sb[:, tb, ob * P:(ob + 1) * P],
                                         rhs=wT1[:, tb, ssl],
                                         start=(tb == 0), stop=(tb == SB - 1))
                    if ob % 2 == 0:
                        nc.scalar.copy(raw[:, ob, ssl], ps_o[:])
                        nc.vector.tensor_scalar(raw8[:, ob, ssl], ps_o[:],
                                                scalar1=float(KAPS),
                                                scalar2=None, op0=OP.mult)
                    else:
                        nc.vector.tensor_copy(raw[:, ob, ssl], ps_o[:])
                        nc.scalar.activation(raw8[:, ob, ssl], ps_o[:],
                                             AF.Identity, scale=float(KAPS))
                    nc.scalar.activation(sq8[:, ob, :], raw8[:, ob, ssl],
                                         AF.Square)
                ps_mu = psZ.tile([P, HALF], FP, tag="z")
                for ko in range(KO):
                    nc.tensor.matmul(ps_mu[:], lhsT=ones8[:, 0, :],
                                     rhs=raw8[:, ko, ssl],
                                     start=(ko == 0), stop=(ko == KO - 1))
                # mu_t: x-domain mean (for centering); mu_b: y-domain (bf16
                # row 0 for the layer-2 K=1 correction); muy: y-domain fp32.
                nc.scalar.mul(mu_t[:, sh, :], ps_mu[:], float(1.0 / (E * KAPS)))
                muy = lnp.tile([P, HALF], FP, tag="muy")
                nc.vector.tensor_scalar(muy[:], ps_mu[:],
                                        scalar1=float(1.0 / E), scalar2=None,
                                        op0=OP.mult)
                nc.vector.tensor_scalar(mu_b[0:1, sh, :], ps_mu[0:1, :],
                                        scalar1=float(1.0 / E), scalar2=None,
                                        op0=OP.mult)
                ps_sq = psZ.tile([P, HALF], FP, tag="z")
                for ko in range(KO):
                    nc.tensor.matmul(ps_sq[:], lhsT=ones8[:, 0, :],
                                     rhs=sq8[:, ko, :],
                                     start=(ko == 0), stop=(ko == KO - 1))
                ps_z = z_replicated(zpart)
                # z scaled by sqrt(eps)*kappa_s; eps*(kappa_s*z)^2 is then a
                # Square on the scalar engine.
                z_sb = zsbp.tile([P, HALF], FP, tag="zsb", bufs=1)
                nc.scalar.mul(z_sb[:], ps_z[:], float(math.sqrt(EPS) * KAPS))
                zq = lnp.tile([P, HALF], FP, tag="zq")
                nc.scalar.activation(zq[:], z_sb[:], AF.Square)
                var = lnp.tile([P, HALF], FP, tag="var")
                nc.vector.tensor_tensor(var[:], muy[:], muy[:], OP.mult)
                nc.vector.scalar_tensor_tensor(var[:], ps_sq[:],
                                               float(1.0 / E), var[:],
                                               op0=OP.mult, op1=OP.subtract)
                nc.vector.tensor_tensor(var[:], var[:], zq[:], OP.add)
                nc.scalar.activation(var[:], var[:], AF.Sqrt)
                # sd -> column form now; the reciprocal happens later on the
                # tiny [P, SB] column tile (203ns) instead of a 3.3us DVE
                # reciprocal of the full [P, HALF] row form.
                ps_t = psZ.tile([P, HALF], FP, tag="z")
                for sbb in range(4):
                    nc.tensor.transpose(ps_t[:, sbb * P:(sbb + 1) * P],
                                        var[:, sbb * P:(sbb + 1) * P],
                                        identF[:])
                    nc.vector.tensor_copy(
                        sdc[:, sh * 4 + sbb:sh * 4 + sbb + 1],
                        ps_t[:, sbb * P:sbb * P + 1])
                # mean-center the bf16 raw in place (*rstd deferred)
                for ob in range(KO):
                    eng = nc.vector if ob % 2 == 0 else nc.gpsimd
                    eng.tensor_tensor(raw[:, ob, ssl], raw[:, ob, ssl],
                                      mu_t[:, sh, :], OP.subtract)
            return raw, raw8, mu_b, sdc

        def rstd_finalize(sdc):
            """From the sd column tile: reciprocal (tiny), the exp2/v2 scale
            columns, and the replicated-row rstd_s for the q2 multiply (per
            s-block: PE transpose of one rstd column -> [1,128] row -> K=1
            ones-matmul replicates it to all 128 partitions).  Returned as
            interleave callbacks for the surrounding out2 stream."""
            st = {}

            def f_cols():
                rstdc = rcp.tile([P, SB], FP, tag="rstdc", name="rstdc")
                nc.vector.reciprocal(rstdc[:], sdc[:])
                rstdc32 = rcp.tile([P, SB], FP, tag="r32", name="rstdc32")
                nc.vector.tensor_scalar(rstdc32[:], rstdc[:],
                                        scalar1=float(1.0 / SCALE),
                                        scalar2=None, op0=OP.mult)
                rstdcv = rcp.tile([P, SB], FP, tag="rv", name="rstdcv")
                nc.vector.tensor_scalar(rstdcv[:], rstdc[:],
                                        scalar1=float(KAPS), scalar2=None,
                                        op0=OP.mult)
                rowb = rcp.tile([1, SB, P], BF, tag="rowb", name="rowb")
                for half in range(2):
                    ps_t = psZ.tile([P, HALF], FP, tag="z", name="psrow")
                    for j in range(4):
                        idx = half * 4 + j
                        nc.tensor.transpose(ps_t[0:1, j * P:(j + 1) * P],
                                            rstdc[:, idx:idx + 1], identF[:])
                        nc.vector.tensor_copy(rowb[0:1, idx, :],
                                              ps_t[0:1, j * P:(j + 1) * P])
                st.update(rstdc32=rstdc32, rstdcv=rstdcv, rowb=rowb)

            def f_rows():
                rstd_t = rstp.tile([P, 2, HALF], FP, tag="rstd_t",
                                   name="rstd_t")
                rowb = st["rowb"]
                for sh in range(2):
                    ps_r = psZ.tile([P, HALF], FP, tag="z", name="psrep")
                    for blk in range(4):
                        nc.tensor.matmul(ps_r[:, blk * P:(blk + 1) * P],
                                         lhsT=onesB[0:1, :],
                                         rhs=rowb[0:1, sh * 4 + blk, :],
                                         start=True, stop=True)
                    nc.scalar.copy(rstd_t[:, sh, :], ps_r[:, :])
                st["rstd_t"] = rstd_t

            return {2: f_cols, 6: f_rows}, st

        # ---------------- layer-2 out (fp8 DoubleRow) + z + DMA --------------
        # The z pipeline alternates DVE and PE work with long serial
        # latency; emitted standalone it idles the PE ~9us per head.  It is
        # split into three steps interleaved into the NEXT head's out1
        # groups (aux mechanism above); out2_mm then only needs invzc.
        def z2_steps(wT2):
            st = {}

            def s_tree():
                st["zp0"] = z_tree(wT2, 0, "b")
                st["zp1"] = z_tree(wT2, 1, "b")

            def s_rep():
                st["zc"] = stp.tile([P, SB], FP, tag="zc", name="zc2")
                for sh in range(2):
                    ps_z = z_replicated(st[f"zp{sh}"])
                    z_sb = zsbp.tile([P, HALF], FP, tag="zsb2",
                                      name=f"zsb2_{sh}")
                    nc.scalar.copy(z_sb[:], ps_z[:])
                    st[f"zsb{sh}"] = z_sb

            def s_cols():
                zc = st["zc"]
                for sh in range(2):
                    ps_t = psZ.tile([P, HALF], FP, tag="z")
                    for sbb in range(4):
                        nc.tensor.transpose(ps_t[:, sbb * P:(sbb + 1) * P],
                                            st[f"zsb{sh}"][:, sbb * P:(sbb + 1) * P],
                                            identF[:])
                        nc.vector.tensor_copy(
                            zc[:, sh * 4 + sbb:sh * 4 + sbb + 1],
                            ps_t[:, sbb * P:sbb * P + 1])
                invzc = stp.tile([P, SB], FP, tag="invzc",
                                  name="invzc2")
                nc.vector.reciprocal(invzc[:], zc[:])
                st["invzc"] = invzc

            return {0: s_tree, 4: s_rep, 9: s_cols}, st

        def out2_mm(head, wT2, v2_sb, st, aux=None):
            aux = aux or {}
            invzc = st["invzc"]
            for blk in range(SB):
                bsl = slice(blk * P, (blk + 1) * P)
                for nb in range(2):
                    if blk * 2 + nb in aux:
                        aux[blk * 2 + nb]()
                    nsl = slice(nb * HALF, (nb + 1) * HALF)
                    ps_o = psB.tile([P, HALF], FP, tag="big")
                    for tb in range(SB):
                        nc.tensor.matmul(ps_o[:],
                                         lhsT=wT2[:, tb, bsl],
                                         rhs=v2_sb[:, tb, nsl],
                                         start=(tb == 0), stop=(tb == SB - 1))
                    ot = otp.tile([P, HALF], FP, tag="ot")
                    if (blk + nb) % 2 == 0:
                        nc.scalar.activation(ot[:], ps_o[:], AF.Identity,
                                             scale=invzc[:, blk:blk + 1])
                    else:
                        nc.vector.tensor_scalar_mul(ot[:], ps_o[:],
                                                    invzc[:, blk:blk + 1])
                    nc.sync.dma_start(out_d.ap()[head, bsl, nsl], ot[:])

        # ------------- per-head loop, layer-2 pipelined one head back --------
        def wv_fetch(layer, h):
            wv_sb = wvp.tile([P, KO, E], BF, tag="wv", name=f"wv{layer}_{h}")
            nc.sync.dma_start(
                wv_sb[:],
                wv.ap()[layer, h].rearrange("(ko p) o -> p ko o", p=P))
            return wv_sb

        def l1_mid(h, wv_sb):
            qkT, kqT = qk_chain(0, h, hn0T8, None, None)
            wT1 = wt1p.tile([P, SB, S], F8, tag="wT1")
            v_sb = vp.tile([P, SB, E], BF, tag="v")
            scores_v(0, h, qkT, kqT, hn0T, wT1, v_sb, wv_sb, None, None)
            return wT1, v_sb

        def l2_mid(h, st, rst, wv_sb):
            raw, raw8, mu_b, _sdc = st
            qkT2, kqT2 = qk_chain(1, h, raw8, mu_b, rst["rstd_t"])
            wT2 = wt2p.tile([P, SB, S], F8, tag="wT2")
            v2_sb = v2p.tile([P, SB, E], F8, tag="v2")
            scores_v(1, h, qkT2, kqT2, raw, wT2, v2_sb, wv_sb,
                     rst["rstdc32"], rst["rstdcv"])
            return wT2, v2_sb

        pending = None
        for h in range(NH):
            with nc.named_scope(f"l1mid{h}"):
                wT1, v_sb = l1_mid(h, wv_fetch(0, h))
            if pending is not None:
                ph, pst, prst = pending
                with nc.named_scope(f"l2mid{ph}"):
                    wT2, v2_sb = l2_mid(ph, pst, prst, wv_fetch(1, ph))
                aux, zst = z2_steps(wT2)
            else:
                aux = None
            with nc.named_scope(f"o1st{h}"):
                st = out1_stats(h, wT1, v_sb, aux=aux)
            raux, rst = rstd_finalize(st[3])
            if pending is not None:
                with nc.named_scope(f"o2mm{ph}"):
                    out2_mm(ph, wT2, v2_sb, zst, aux=raux)
            else:
                # head 0: no out2 stream to interleave into
                for fn in raux.values():
                    fn()
            pending = (h, st, rst)
        # drain: last head's layer 2 (z pipeline latency exposed once)
        ph, pst, prst = pending
        wT2, v2_sb = l2_mid(ph, pst, prst, wv_fetch(1, ph))
        aux, zst = z2_steps(wT2)
        for fn in aux.values():
            fn()
        out2_mm(ph, wT2, v2_sb, zst)

    if legalize:
        _legalize_multi_waits(nc)
    return nc


_CACHE = {}


def _get_nc(g0_identity, g1_identity, legalize=True):
    key = (g0_identity, g1_identity, legalize)
    if key not in _CACHE:
        _CACHE[key] = _build_nc(g0_identity, g1_identity, legalize)
    return _CACHE[key]


def _prep_in_maps(x, emb, ln_gamma, ln_beta, Wq, Wk, Wv):
    x = np.asarray(x)
    bf = ml_dtypes.bfloat16
    f8 = ml_dtypes.float8_e4m3
    emb = np.ascontiguousarray(np.asarray(emb, dtype=np.float32).astype(bf))
    ln_gamma = np.asarray(ln_gamma, dtype=np.float32)
    ln_beta = np.asarray(ln_beta, dtype=np.float32)
    Wq = np.asarray(Wq, dtype=np.float32)
    Wk = np.asarray(Wk, dtype=np.float32)
    Wv = np.asarray(Wv, dtype=np.float32)

    # [L,H,E,2A] packed (WqT | WkT) in fp8e4; [L,H,E,E] = WvT in bf16
    wqkT = np.concatenate([Wq.transpose(0, 1, 3, 2), Wk.transpose(0, 1, 3, 2)],
                          axis=3)
    wqkT8 = np.clip(wqkT, -240, 240).astype(f8)
    wvT = Wv.transpose(0, 1, 3, 2).astype(bf)
    # layer-2 mean-correction row: -(sum_e Wq2 | sum_e Wk2), consistent
    # with the fp8 weights actually used in the matmul.
    w1 = -wqkT8[1].astype(np.float32).sum(axis=1).astype(bf)  # [H, 2A]

    in_maps = []
    for c in range(8):
        b = c // 2
        hs = (c % 2) * NH
        in_maps.append({
            "emb": emb,
            "xidx": np.ascontiguousarray(x[b].astype(np.int32).reshape(S, 1)),
            "wqk8": np.ascontiguousarray(wqkT8[:, hs:hs + NH]),
            "wv": np.ascontiguousarray(wvT[:, hs:hs + NH]),
            "negw1": np.ascontiguousarray(w1[hs:hs + NH]),
            "g0": np.ascontiguousarray(ln_gamma[0]),
            "b0": np.ascontiguousarray(ln_beta[0]),
            "g1": np.ascontiguousarray(ln_gamma[1]),
            "b1": np.ascontiguousarray(ln_beta[1]),
        })
    g0_id = bool(np.all(ln_gamma[0] == 1.0) and np.all(ln_beta[0] == 0.0))
    g1_id = bool(np.all(ln_gamma[1] == 1.0) and np.all(ln_beta[1] == 0.0))
    return in_maps, g0_id, g1_id


def run(inputs, trace=False, trace_cores=None):
    in_maps, g0_id, g1_id = _prep_in_maps(**inputs)
    nc = _get_nc(g0_id, g1_id)
    res = run_bass_kernel_spmd(nc, in_maps, core_ids=list(range(8)),
                               trace=trace, trace_cores=trace_cores)
    out = np.empty((B, H, S, E), dtype=np.float32)
    for c in range(8):
        out[c // 2, (c % 2) * NH:(c % 2) * NH + NH] = res.results[c]["out"]
    return out, res


def kernel(x, emb, ln_gamma, ln_beta, Wq, Wk, Wv):
    out, _ = run(dict(x=x, emb=emb, ln_gamma=ln_gamma, ln_beta=ln_beta,
                      Wq=Wq, Wk=Wk, Wv=Wv))
    return out
